# revision 9
# baseline (speedup 1.0000x reference)
"""Trainium2 Bass kernel for nn_CoarsePyramid (nms_detection).

Data-parallel over batch: B=8 -> 8 NeuronCores, one batch element each.

Per-core pipeline (C=512, T=64, TF=256, CCONF=400, GROUPS=32):
  fm_short = CGR(feature, w_cur)            [512, 64]
  feat2    = CGR(feature, w_lr)             [1024, 64]   (also an output)
  prop_feature = boundary_pool(feat2, segments)          [1024, 64]
  prop_roi = CGR(boundary_pool(flf, frame_segments), w_roi)  [512, 64]
  out = CGR(cat(prop_roi, prop_feature, fm_short, conf), w_prop)  [512, 64]

conv1x1 = PE matmuls with host-pre-transposed weights ([Cin, Cout]) and the
bias folded in as one extra contraction row against a ones row. All four
output tiles of one conv share one PSUM bank. GroupNorm stats (sum, sum of
squares) come from one DVE segmented reduce + one ACT Square pass, get
group-summed and broadcast back through tiny PE matmuls with 0/1 group
masks, and are applied fused with ReLU via ACT(Relu, scale=A, bias=B).
Boundary max pooling: sparse max-tables at levels {0,2,4(,6)} built with DVE
shifted-max; queries are 4 idempotent anchors per segment gathered on
GPSIMD ap_gather with host-precomputed int16 indices; empty segments are
zeroed with a broadcast 0/1 mask.
"""

import contextlib

import numpy as np
import ml_dtypes

import concourse.bass as bass
import concourse.bacc as bacc
import concourse.tile as tile
import concourse.mybir as mybir
from concourse import bass_utils

B, C, T, TF, CCONF = 8, 512, 64, 256, 400
GROUPS, EPS = 32, 1e-5
F32 = mybir.dt.float32
BF16 = mybir.dt.bfloat16
I16 = mybir.dt.int16
AF = mybir.ActivationFunctionType
ALU = mybir.AluOpType
AX = mybir.AxisListType

N_SEG = 64           # proposals per batch element
LEV2 = (0, 2, 4)     # table levels for feat2 pooling (Tin=64)
LEVF = (0, 2, 4, 6)  # table levels for frame pooling (Tin=256)
NANCH = 4
NT = 4               # output tiles per conv call (Cout=512 per call)

_COMPILED = {}


# --------------------------------------------------------------------------
# host-side input prep
# --------------------------------------------------------------------------

def _wt_pad(w, b):
    """[Cout, Cin] weight + [Cout] bias -> [Cin+1, Cout] f32 (bias = last row)."""
    return np.concatenate([w.T, b[None, :]], axis=0).astype(np.float32).copy()


def _gb(g, be):
    """gamma/beta [512] -> [128, 8]: cols [0:4] gamma tiles, [4:8] beta."""
    gt = g.reshape(NT, 128).T
    bt = be.reshape(NT, 128).T
    return np.concatenate([gt, bt], axis=1).astype(np.float32).copy()


def _pool_idx_mask(seg, tin, levels, ntiles):
    """Anchor gather indices + empty mask for one batch element.

    seg: [N, 4] raw float segment bounds (cols: left lo/hi, right lo/hi).
    Returns idx [128, ntiles*NANCH*N_SEG//16] int16 (ap_gather wrapped,
    per-tile blocks of NANCH*N indices into that tile's [nlev*tin] table)
    and mask [1, ntiles*N_SEG] bf16 (0 for empty segments).
    """
    s = np.clip(np.floor(seg), 0, tin - 1).astype(np.int64)  # [N, 4]
    half_idx, half_msk = [], []
    for h in range(2):
        lo, hi = s[:, 2 * h], s[:, 2 * h + 1]
        ln = hi - lo + 1
        ok = ln >= 1
        ln_c = np.maximum(ln, 1)
        # smallest level k in `levels` with NANCH * 2^k >= len
        k = np.full_like(ln_c, levels[0])
        slot = np.zeros_like(ln_c)
        for si, kk in enumerate(levels[:-1]):
            sel = ln_c > NANCH * (2 ** kk)
            k = np.where(sel, levels[si + 1], k)
            slot = np.where(sel, si + 1, slot)
        step = 2 ** k
        anchors = []
        for i in range(NANCH):
            a = np.minimum(lo + i * step, hi - step + 1)
            a = np.clip(a, 0, tin - 1)
            anchors.append(slot * tin + a)
        idx = np.stack(anchors, axis=0)          # [NANCH, N]
        half_idx.append(np.where(ok[None, :], idx, 0))
        half_msk.append(ok.astype(np.float32))
    per_tile, msk = [], []
    for j in range(ntiles):
        h = 0 if j < ntiles // 2 else 1
        per_tile.append(half_idx[h].reshape(-1))  # [NANCH*N]
        msk.append(half_msk[h])
    idx_flat = np.concatenate(per_tile)           # [ntiles*NANCH*N]
    wrapped = idx_flat.reshape(ntiles, NANCH * N_SEG // 16, 16)
    wrapped = np.transpose(wrapped, (2, 0, 1)).reshape(16, -1)  # [16, nt*16]
    idx16 = np.tile(wrapped, (8, 1)).astype(np.int16).copy()    # [128, nt*16]
    mask = np.concatenate(msk)[None, :].astype(ml_dtypes.bfloat16).copy()
    return idx16, mask


def _host_prep(inputs):
    f = {k: np.asarray(v) for k, v in inputs.items()}
    shared = {
        "wt_cur": _wt_pad(f["w_cur"], f["b_cur"]),
        "wt_lr": _wt_pad(f["w_lr"], f["b_lr"]),
        "wt_roi": _wt_pad(f["w_roi"], f["b_roi"]),
        "wt_prop": _wt_pad(f["w_prop"], f["b_prop"]),
        "gb_cur": _gb(f["g_cur"], f["be_cur"]),
        "gb_lr_a": _gb(f["g_lr"][:512], f["be_lr"][:512]),
        "gb_lr_b": _gb(f["g_lr"][512:], f["be_lr"][512:]),
        "gb_roi": _gb(f["g_roi"], f["be_roi"]),
        "gb_prop": _gb(f["g_prop"], f["be_prop"]),
    }
    p = np.arange(128)
    shared["gmask16"] = (p[:, None] // 16 == np.arange(8)[None, :]).astype(np.float32)
    shared["gmask16T"] = shared["gmask16"].T.copy()
    shared["gmask32"] = (p[:, None] // 32 == np.arange(4)[None, :]).astype(np.float32)
    shared["gmask32T"] = shared["gmask32"].T.copy()
    shared["ones1b"] = np.ones((1, 128), dtype=ml_dtypes.bfloat16)

    in_maps = []
    for b in range(B):
        m = dict(shared)
        m["x0"] = np.ascontiguousarray(f["feature"][b]).astype(np.float32)
        m["flf"] = np.ascontiguousarray(f["frame_level_feature"][b]).astype(np.float32)
        m["conf"] = np.ascontiguousarray(f["conf_result_feature"][b]).astype(np.float32)
        m["idx2"], m["em2"] = _pool_idx_mask(f["segments"][b], T, LEV2, 8)
        m["idxf"], m["emf"] = _pool_idx_mask(f["frame_segments"][b], TF, LEVF, 4)
        in_maps.append(m)
    return in_maps


# --------------------------------------------------------------------------
# device kernel
# --------------------------------------------------------------------------

def _conv_gn_relu(tc, pools, wt, wbias, gb, gmask, gmaskT, rhs_tiles, nkt,
                  cnt, out_writes, ones_row, zcol, epscol, m0=0, last_k=None):
    """conv1x1 (+bias) -> GroupNorm -> ReLU for NT=4 output tiles.

    wt: SBUF [128, nkt, Cout_total]; output cols (m0+m)*128. wbias is a
    [1, Cout_total] row; bias lands via a K=1 matmul against ones_row.
    last_k: partition rows of the final k-tile (e.g. 16 for the conf tail).
    cnt: elements per group (group_size * T).
    """
    nc = tc.nc
    sb = pools["sbuf_small"]
    ps = pools["psum"].tile([128, NT * T], F32, tag="conv_ps")
    for m in range(NT):
        for k in range(nkt):
            kk = 128 if (last_k is None or k < nkt - 1) else last_k
            nc.tensor.matmul(
                ps[:, bass.ts(m, T)],
                wt[0:kk, k, bass.ts(m0 + m, 128)],
                rhs_tiles[k][0:kk, :],
                start=(k == 0),
                stop=False,
            )
        nc.tensor.matmul(
            ps[:, bass.ts(m, T)],
            wbias[0:1, bass.ts(m0 + m, 128)],
            ones_row[0:1, :],
            start=False, stop=True,
        )

    # per-channel sum + sum of squares
    s_ss = sb.tile([128, 2 * NT], F32, tag="s_ss")
    nc.vector.tensor_reduce(
        s_ss[:, 0:NT], ps[:].rearrange("p (m t) -> p m t", m=NT),
        axis=AX.X, op=ALU.add)
    sq = pools["scratch"].tile([128, NT * T], F32, tag="sq")
    nc.scalar.activation(sq[:], ps[:], AF.Square, bias=zcol[:, 0:1])
    nc.vector.tensor_reduce(
        s_ss[:, NT : 2 * NT], sq[:].rearrange("p (m t) -> p m t", m=NT),
        axis=AX.X, op=ALU.add)

    G = gmask.shape[-1]
    st_ps = pools["psum_small"].tile([G, 2 * NT], F32, tag="st_ps")
    nc.tensor.matmul(st_ps[:], gmask[:], s_ss[:], start=True, stop=True)
    mu_rs = sb.tile([G, 2 * NT], F32, tag="mu_rs")
    # mu = sum/cnt ; var = sumsq/cnt - mu^2 ; rs = 1/sqrt(var+eps)
    nc.vector.tensor_scalar_mul(mu_rs[:, 0:NT], st_ps[:, 0:NT], 1.0 / cnt)
    var = sb.tile([G, NT], F32, tag="var")
    nc.vector.tensor_scalar_mul(var[:], st_ps[:, NT:], 1.0 / cnt)
    mu2 = sb.tile([G, NT], F32, tag="mu2")
    nc.vector.tensor_tensor(mu2[:], mu_rs[:, 0:NT], mu_rs[:, 0:NT], ALU.mult)
    nc.vector.tensor_tensor(var[:], var[:], mu2[:], ALU.subtract)
    sd = sb.tile([G, NT], F32, tag="sd")
    nc.scalar.activation(sd[:], var[:], AF.Sqrt, bias=epscol[0:G, 0:1])
    nc.vector.reciprocal(mu_rs[:, NT:], sd[:])

    bc_ps = pools["psum_small"].tile([128, 2 * NT], F32, tag="bc_ps")
    nc.tensor.matmul(bc_ps[:], gmaskT[:], mu_rs[:], start=True, stop=True)
    mb = sb.tile([128, 2 * NT], F32, tag="mb")
    nc.scalar.copy(mb[:], bc_ps[:])
    # A = rs*gamma ; Bv = beta - mu*A
    a_b = sb.tile([128, 2 * NT], F32, tag="a_b")
    nc.vector.tensor_tensor(a_b[:, 0:NT], mb[:, NT:], gb[:, 0:NT], ALU.mult)
    tmp = sb.tile([128, NT], F32, tag="abtmp")
    nc.vector.tensor_tensor(tmp[:], mb[:, 0:NT], a_b[:, 0:NT], ALU.mult)
    nc.vector.tensor_tensor(a_b[:, NT:], gb[:, NT:], tmp[:], ALU.subtract)

    for m in range(NT):
        nc.scalar.activation(
            out_writes[m], ps[:, bass.ts(m, T)], AF.Relu,
            scale=a_b[:, m : m + 1], bias=a_b[:, NT + m : NT + m + 1])


def _build_tables(nc, pools, tbl, ntiles, tin, levels):
    """Build max-tables in tbl [128, ntiles, nlev, tin] (slot 0 = raw data).

    Level k is built from level k-2 via two shifted-max passes (intermediate
    k-1 in scratch). Valid width of level k is tin - 2^k + 1; tails stay
    garbage and are never queried.
    """
    for si in range(1, len(levels)):
        k = levels[si]
        d1, w1 = 2 ** (k - 2), tin - 2 ** (k - 1) + 1
        tmp = pools["scratch_tbl"].tile([128, ntiles, tin], F32, tag=f"ttmp{tin}")
        nc.vector.tensor_tensor(
            tmp[:, :, 0:w1],
            tbl[:, :, si - 1, 0:w1],
            tbl[:, :, si - 1, d1 : d1 + w1],
            ALU.max)
        d2, w2 = 2 ** (k - 1), tin - 2 ** k + 1
        nc.vector.tensor_tensor(
            tbl[:, :, si, 0:w2], tmp[:, :, 0:w2], tmp[:, :, d2 : d2 + w2], ALU.max)


def _pool_query(tc, pools, tbl, idx, em, ones1b, ntiles, tin, nlev, out):
    """Gather 4 anchors per proposal per tile, max them, zero empty segs."""
    nc = tc.nc
    gout = pools["gout"].tile([128, ntiles, NANCH, N_SEG], F32, tag=f"gout{tin}")
    for j in range(ntiles):
        nc.gpsimd.ap_gather(
            gout[:, j, :, :],
            tbl[:, j, :, :],
            idx[:, j * 16 : (j + 1) * 16],
            channels=128, num_elems=nlev * tin, d=1, num_idxs=NANCH * N_SEG)
    m1 = pools["scratch"].tile([128, ntiles, N_SEG], F32, tag=f"pm1_{tin}")
    m2 = pools["scratch"].tile([128, ntiles, N_SEG], F32, tag=f"pm2_{tin}")
    nc.vector.tensor_tensor(m1[:], gout[:, :, 0, :], gout[:, :, 1, :], ALU.max)
    nc.vector.tensor_tensor(m2[:], gout[:, :, 2, :], gout[:, :, 3, :], ALU.max)
    nc.vector.tensor_tensor(m1[:], m1[:], m2[:], ALU.max)
    # zero empty segments: broadcast [1, ntiles*N] bf16 mask to 128 partitions
    mps = pools["psum_small"].tile([128, 8 * N_SEG], F32, tag="mps")
    nc.tensor.matmul(mps[:, 0 : ntiles * N_SEG], ones1b[:], em[:],
                     start=True, stop=True)
    nc.vector.tensor_tensor(
        out[:], m1[:],
        mps[:, 0 : ntiles * N_SEG].rearrange("p (j n) -> p j n", j=ntiles),
        ALU.mult)


def _build_nc():
    nc = bacc.Bacc("TRN2", target_bir_lowering=False, debug=False,
                   enable_asserts=False, num_devices=B)

    din = {}
    def dram_in(name, shape, dtype=F32):
        din[name] = nc.dram_tensor(name, list(shape), dtype,
                                   kind="ExternalInput").ap()
        return din[name]

    x0 = dram_in("x0", (C, T))
    flf = dram_in("flf", (C, TF))
    conf = dram_in("conf", (CCONF, T))
    wt_cur = dram_in("wt_cur", (C + 1, C))
    wt_lr = dram_in("wt_lr", (C + 1, 2 * C))
    wt_roi = dram_in("wt_roi", (C + 1, C))
    wt_prop = dram_in("wt_prop", (4 * C + CCONF + 1, C))
    for nm, sh in [("gb_cur", (128, 8)), ("gb_lr_a", (128, 8)),
                   ("gb_lr_b", (128, 8)), ("gb_roi", (128, 8)),
                   ("gb_prop", (128, 8)), ("gmask16", (128, 8)),
                   ("gmask16T", (8, 128)), ("gmask32", (128, 4)),
                   ("gmask32T", (4, 128))]:
        dram_in(nm, sh)
    dram_in("ones1b", (1, 128), BF16)
    dram_in("idx2", (128, 128), I16)
    dram_in("em2", (1, 8 * N_SEG), BF16)
    dram_in("idxf", (128, 64), I16)
    dram_in("emf", (1, 4 * N_SEG), BF16)

    out_d = nc.dram_tensor("out", [C, T], F32, kind="ExternalOutput").ap()
    feat2_d = nc.dram_tensor("feat2", [2 * C, T], F32, kind="ExternalOutput").ap()

    with tile.TileContext(nc) as tc, contextlib.ExitStack() as ctx:
        pools = {
            "consts": ctx.enter_context(tc.tile_pool(name="consts", bufs=1)),
            "wts": ctx.enter_context(tc.tile_pool(name="wts", bufs=1)),
            "acts": ctx.enter_context(tc.tile_pool(name="acts", bufs=1)),
            "sbuf_small": ctx.enter_context(tc.tile_pool(name="sbs", bufs=2)),
            "scratch": ctx.enter_context(tc.tile_pool(name="scr", bufs=2)),
            "scratch_tbl": ctx.enter_context(tc.tile_pool(name="scrt", bufs=2)),
            "gout": ctx.enter_context(tc.tile_pool(name="gout", bufs=1)),
            "psum": ctx.enter_context(
                tc.tile_pool(name="psum", bufs=4, space="PSUM")),
            "psum_small": ctx.enter_context(
                tc.tile_pool(name="psums", bufs=1, space="PSUM")),
        }
        co, wp, ap_ = pools["consts"], pools["wts"], pools["acts"]

        def load(pool, name, shape, dtype=F32, src_ap=None):
            t = pool.tile(list(shape), dtype, tag=name)
            nc.sync.dma_start(t[:], src_ap if src_ap is not None else din[name][:])
            return t

        # ---- constants / small inputs
        GB_CUR = load(co, "gb_cur", (128, 8))
        GB_LR_A = load(co, "gb_lr_a", (128, 8))
        GB_LR_B = load(co, "gb_lr_b", (128, 8))
        GB_ROI = load(co, "gb_roi", (128, 8))
        GB_PROP = load(co, "gb_prop", (128, 8))
        GM16 = load(co, "gmask16", (128, 8))
        GM16T = load(co, "gmask16T", (8, 128))
        GM32 = load(co, "gmask32", (128, 4))
        GM32T = load(co, "gmask32T", (4, 128))
        ONES1B = load(co, "ones1b", (1, 128), BF16)
        IDX2 = load(co, "idx2", (128, 128), I16)
        EM2 = load(co, "em2", (1, 8 * N_SEG), BF16)
        IDXF = load(co, "idxf", (128, 64), I16)
        EMF = load(co, "emf", (1, 4 * N_SEG), BF16)

        ones_row = co.tile([1, T], F32, tag="ones_row")
        nc.vector.memset(ones_row[:], 1.0)
        zcol = co.tile([128, 1], F32, tag="zcol")
        nc.vector.memset(zcol[:], 0.0)
        epscol = co.tile([8, 1], F32, tag="epscol")
        nc.vector.memset(epscol[:], EPS)

        # ---- activations in
        X0 = load(ap_, "x0", (128, 4, T),
                  src_ap=x0.rearrange("(j p) t -> p j t", p=128))

        TBLF = ap_.tile([128, 4, len(LEVF), TF], F32, tag="tblf")
        nc.sync.dma_start(TBLF[:, :, 0, :],
                          flf.rearrange("(j p) t -> p j t", p=128))

        CONF = ap_.tile([128, 4, T], F32, tag="conf_t")
        nc.sync.dma_start(CONF[:, 0:3, :],
                          conf[0 : 3 * 128, :].rearrange("(j p) t -> p j t", p=128))
        nc.sync.dma_start(CONF[0:16, 3, :], conf[3 * 128 :, :])

        # ---- weights, k-major tiles [128, nkt(+1), Cout]
        def load_wt(name, dram, nkt, cout):
            wt_t = wp.tile([128, nkt, cout], F32, tag=name)
            full = (nkt - 1) if nkt * 128 > dram.shape[0] - 1 else nkt
            nc.sync.dma_start(
                wt_t[:, 0:full, :],
                dram[0 : full * 128, :].rearrange("(k p) o -> p k o", p=128))
            if full < nkt:
                rem = dram.shape[0] - 1 - full * 128
                nc.sync.dma_start(wt_t[0:rem, full, :],
                                  dram[full * 128 : full * 128 + rem, :])
            wb = wp.tile([1, cout], F32, tag=name + "_b")
            nc.sync.dma_start(wb[:], dram[dram.shape[0] - 1 :, :])
            return wt_t, wb

        WT_CUR, WB_CUR = load_wt("wt_cur", wt_cur, 4, C)
        WT_LR, WB_LR = load_wt("wt_lr", wt_lr, 4, 2 * C)
        WT_ROI, WB_ROI = load_wt("wt_roi", wt_roi, 4, C)
        WT_PROP, WB_PROP = load_wt("wt_prop", wt_prop, 20, C)

        # ---- frame pooling path (independent of the conv chain)
        _build_tables(nc, pools, TBLF, 4, TF, LEVF)
        PR = ap_.tile([128, 4, T], F32, tag="pr")
        _pool_query(tc, pools, TBLF, IDXF, EMF, ONES1B, 4, TF, len(LEVF), PR)

        # ---- conv_cur
        FMS = ap_.tile([128, 4, T], F32, tag="fms")
        x0_tiles = [X0[:, k, :] for k in range(4)]
        _conv_gn_relu(tc, pools, WT_CUR, WB_CUR, GB_CUR, GM16, GM16T, x0_tiles,
                      4, 16 * T, [FMS[:, m, :] for m in range(4)], ones_row, zcol, epscol)

        # ---- conv_lr -> feat2 (table slot 0), two halves of 512 channels
        TBL2 = ap_.tile([128, 8, len(LEV2), T], F32, tag="tbl2")
        _conv_gn_relu(tc, pools, WT_LR, WB_LR, GB_LR_A, GM32, GM32T, x0_tiles,
                      4, 32 * T, [TBL2[:, m, 0, :] for m in range(4)], ones_row, zcol, epscol)
        _conv_gn_relu(tc, pools, WT_LR, WB_LR, GB_LR_B, GM32, GM32T, x0_tiles,
                      4, 32 * T, [TBL2[:, 4 + m, 0, :] for m in range(4)],
                      ones_row, zcol, epscol, m0=4)
        nc.sync.dma_start(feat2_d.rearrange("(j p) t -> p j t", p=128),
                          TBL2[:, :, 0, :])

        # ---- feat2 pooling
        _build_tables(nc, pools, TBL2, 8, T, LEV2)
        PF = ap_.tile([128, 8, T], F32, tag="pf")
        _pool_query(tc, pools, TBL2, IDX2, EM2, ONES1B, 8, T, len(LEV2), PF)

        # ---- conv_roi on pooled frames
        ROIC = ap_.tile([128, 4, T], F32, tag="roic")
        _conv_gn_relu(tc, pools, WT_ROI, WB_ROI, GB_ROI, GM16, GM16T,
                      [PR[:, k, :] for k in range(4)],
                      4, 16 * T, [ROIC[:, m, :] for m in range(4)], ones_row, zcol, epscol)

        # ---- conv_prop on the concat (rhs tiles, no physical concat)
        rhs = ([ROIC[:, k, :] for k in range(4)]
               + [PF[:, k, :] for k in range(8)]
               + [FMS[:, k, :] for k in range(4)]
               + [CONF[:, k, :] for k in range(4)])
        OUT = ap_.tile([128, 4, T], F32, tag="out_t")
        _conv_gn_relu(tc, pools, WT_PROP, WB_PROP, GB_PROP, GM16, GM16T, rhs,
                      20, 16 * T, [OUT[:, m, :] for m in range(4)], ones_row,
                      zcol, epscol, last_k=16)
        nc.sync.dma_start(out_d.rearrange("(j p) t -> p j t", p=128), OUT[:])

    nc.compile()
    return nc


# --------------------------------------------------------------------------
# entry point
# --------------------------------------------------------------------------

def kernel(**inputs):
    if "nc" not in _COMPILED:
        _COMPILED["nc"] = _build_nc()
    nc = _COMPILED["nc"]
    in_maps = _host_prep(inputs)
    res = bass_utils.run_bass_kernel_spmd(nc, in_maps, core_ids=list(range(B)))
    outs = res.results
    out = np.stack([outs[b]["out"] for b in range(B)], axis=0)
    feat2 = np.stack([outs[b]["feat2"] for b in range(B)], axis=0)
    return out.astype(np.float32), feat2.astype(np.float32)


# revision 10
# speedup vs baseline: 2.2886x; 2.2886x over previous
"""Trainium2 Bass kernel for nn_CoarsePyramid (nms_detection).

Data-parallel over batch: B=8 -> 8 NeuronCores, one batch element each.

Per-core pipeline (C=512, T=64, TF=256, CCONF=400, GROUPS=32):
  fm_short = CGR(feature, w_cur)            [512, 64]
  feat2    = CGR(feature, w_lr)             [1024, 64]   (also an output)
  prop_feature = boundary_pool(feat2, segments)          [1024, 64]
  prop_roi = CGR(boundary_pool(flf, frame_segments), w_roi)  [512, 64]
  out = CGR(cat(prop_roi, prop_feature, fm_short, conf), w_prop)  [512, 64]

conv1x1: PE matmuls in bf16 hi/lo split (x ~ xh+xl, w ~ wh+wl; psum +=
wh*xh + wh*xl + wl*xh, fp32 accumulate; ~1e-5 rel err) — ~4x faster than
native fp32 matmul on TRN2. Weights pre-transposed on host and shipped as
two bf16 tensors (same total bytes as fp32). Bias rides as an extra
contraction row (K=1 two-pass matmuls against a bf16 ones row; for
conv_prop it sits inside the K=17 conf tail tile).
GroupNorm: per-channel sum (DVE segmented reduce) + sum of squares (ACT
Square + DVE reduce) from PSUM, group-summed/broadcast via tiny fp32 PE
matmuls with 0/1 masks, applied fused with ReLU via ACT(Relu, scale, bias).
Boundary max pooling: full sparse max-table (levels 0..log2(Tin)) built by
DVE shifted-max in a position-major layout packed d-wide over channel
tiles; queries = 2 idempotent anchors per segment, gathered by GPSIMD
ap_gather (one call per half, 128 host-precomputed int16 indices); empty
segments zeroed via a broadcast 0/1 mask.
"""

import contextlib

import numpy as np
import ml_dtypes

import concourse.bass as bass
import concourse.bacc as bacc
import concourse.tile as tile
import concourse.mybir as mybir
from concourse import bass_utils

B, C, T, TF, CCONF = 8, 512, 64, 256, 400
GROUPS, EPS = 32, 1e-5
F32 = mybir.dt.float32
BF16 = mybir.dt.bfloat16
I16 = mybir.dt.int16
AF = mybir.ActivationFunctionType
ALU = mybir.AluOpType
AX = mybir.AxisListType

N_SEG = 64
NLEV2 = 7   # levels 0..6 for Tin=64
NLEVF = 9   # levels 0..8 for Tin=256
NANCH = 2
NT = 4      # output tiles per conv call (Cout=512 per call)

_COMPILED = {}
BF = ml_dtypes.bfloat16


# --------------------------------------------------------------------------
# host-side input prep
# --------------------------------------------------------------------------

def _hi_lo(a):
    hi = a.astype(BF)
    lo = (a - hi.astype(np.float32)).astype(BF)
    return hi, lo


def _wt_pad(w, b):
    """[Cout, Cin] weight + [Cout] bias -> hi/lo bf16 [Cin+1, Cout]."""
    wt = np.concatenate([w.T, b[None, :]], axis=0).astype(np.float32)
    hi, lo = _hi_lo(wt)
    return np.ascontiguousarray(hi), np.ascontiguousarray(lo)


def _gb(g, be):
    """gamma/beta [512] -> [128, 8]: cols [0:4] gamma tiles, [4:8] beta."""
    gt = g.reshape(NT, 128).T
    bt = be.reshape(NT, 128).T
    return np.concatenate([gt, bt], axis=1).astype(np.float32).copy()


def _pool_idx_mask(seg, tin, njj):
    """2-anchor sparse-table gather indices + empty mask.

    Returns idx [128, 16] int16 (two per-half wrapped blocks of
    NANCH*N_SEG indices into that half's [nlev*tin] table) and mask
    [1, 2*njj*N_SEG] bf16 (0 for empty segments), (half, jj)-major.
    """
    s = np.clip(np.floor(seg), 0, tin - 1).astype(np.int64)  # [N, 4]
    idx_h, msk = [], []
    for h in range(2):
        lo, hi = s[:, 2 * h], s[:, 2 * h + 1]
        ln = hi - lo + 1
        ok = ln >= 1
        ln_c = np.maximum(ln, 1)
        k = np.floor(np.log2(ln_c)).astype(np.int64)  # 2^k <= len
        step = 2 ** k
        a0 = k * tin + lo
        a1 = k * tin + np.maximum(hi - step + 1, 0)
        idx = np.stack([a0, a1], axis=0)              # [NANCH, N]
        idx_h.append(np.where(ok[None, :], idx, 0).reshape(-1))
        msk.extend([ok.astype(np.float32)] * njj)
    idx_flat = np.concatenate(idx_h)                  # [2*NANCH*N]
    wrapped = idx_flat.reshape(2, NANCH * N_SEG // 16, 16)
    wrapped = np.transpose(wrapped, (2, 0, 1)).reshape(16, -1)
    idx16 = np.tile(wrapped, (8, 1)).astype(np.int16).copy()  # [128, 16]
    mask = np.concatenate(msk)[None, :].astype(BF).copy()     # [1, 2*njj*N]
    return idx16, mask


def _host_prep(inputs):
    f = {k: np.asarray(v) for k, v in inputs.items()}
    shared = {}
    for nm in ("cur", "lr", "roi", "prop"):
        hi, lo = _wt_pad(f[f"w_{nm}"], f[f"b_{nm}"])
        shared[f"wth_{nm}"], shared[f"wtl_{nm}"] = hi, lo
    shared["gb_cur"] = _gb(f["g_cur"], f["be_cur"])
    shared["gb_lr_a"] = _gb(f["g_lr"][:512], f["be_lr"][:512])
    shared["gb_lr_b"] = _gb(f["g_lr"][512:], f["be_lr"][512:])
    shared["gb_roi"] = _gb(f["g_roi"], f["be_roi"])
    shared["gb_prop"] = _gb(f["g_prop"], f["be_prop"])
    p = np.arange(128)
    shared["gmask16"] = (p[:, None] // 16 == np.arange(8)[None, :]).astype(np.float32)
    shared["gmask16T"] = shared["gmask16"].T.copy()
    shared["gmask32"] = (p[:, None] // 32 == np.arange(4)[None, :]).astype(np.float32)
    shared["gmask32T"] = shared["gmask32"].T.copy()
    shared["ones1b"] = np.ones((1, 128), dtype=BF)
    shared["onezero64"] = np.stack(
        [np.ones(T, np.float32), np.zeros(T, np.float32)]).astype(BF)

    in_maps = []
    for b in range(B):
        m = dict(shared)
        m["x0"] = np.ascontiguousarray(f["feature"][b]).astype(np.float32)
        m["flf"] = np.ascontiguousarray(f["frame_level_feature"][b]).astype(np.float32)
        m["conf"] = np.ascontiguousarray(f["conf_result_feature"][b]).astype(np.float32)
        m["idx2"], m["em2"] = _pool_idx_mask(f["segments"][b], T, 4)
        m["idxf"], m["emf"] = _pool_idx_mask(f["frame_segments"][b], TF, 2)
        in_maps.append(m)
    return in_maps


# --------------------------------------------------------------------------
# device kernel
# --------------------------------------------------------------------------

def _conv_gn_relu(tc, pools, wth, wtl, gb, gmask, gmaskT, rhs_h, rhs_l, nkt,
                  cnt, out_writes, onesb, zcol, epscol, m0=0, last_k=None):
    """bf16 hi/lo conv1x1 (+bias) -> GroupNorm -> ReLU for NT=4 out tiles.

    wth/wtl: SBUF bf16 [128, nkt(+1), Cout_total]; rhs_h/rhs_l: per-k bf16
    [*, T] APs. If last_k is None, bias = K=1 two-pass matmul (wt tile nkt,
    partition 0) against onesb; else the final k-tile has K=last_k rows
    with the bias row included (rhs row last_k-1 is ones in rhs_h and
    zero in rhs_l). cnt: elements per group.
    """
    nc = tc.nc
    sb = pools["sbuf_small"]
    ps = pools["psum"].tile([128, NT * T], F32, tag="conv_ps")
    for m in range(NT):
        out_ap = ps[:, bass.ts(m, T)]
        for k in range(nkt):
            kk = 128 if (last_k is None or k < nkt - 1) else last_k
            wh = wth[0:kk, k, bass.ts(m0 + m, 128)]
            wl = wtl[0:kk, k, bass.ts(m0 + m, 128)]
            last = last_k is not None and k == nkt - 1
            nc.tensor.matmul(out_ap, wh, rhs_h[k][0:kk, :],
                             start=(k == 0), stop=False)
            nc.tensor.matmul(out_ap, wh, rhs_l[k][0:kk, :],
                             start=False, stop=False)
            nc.tensor.matmul(out_ap, wl, rhs_h[k][0:kk, :],
                             start=False, stop=last)
        if last_k is None:
            nc.tensor.matmul(out_ap, wth[0:1, nkt, bass.ts(m0 + m, 128)],
                             onesb[0:1, :], start=False, stop=False)
            nc.tensor.matmul(out_ap, wtl[0:1, nkt, bass.ts(m0 + m, 128)],
                             onesb[0:1, :], start=False, stop=True)

    # per-channel sum + sum of squares
    s_ss = sb.tile([128, 2 * NT], F32, tag="s_ss")
    nc.vector.tensor_reduce(
        s_ss[:, 0:NT], ps[:].rearrange("p (m t) -> p m t", m=NT),
        axis=AX.X, op=ALU.add)
    sq = pools["scratch"].tile([128, NT * T], F32, tag="sq")
    nc.scalar.activation(sq[:], ps[:], AF.Square, bias=zcol[:, 0:1])
    nc.vector.tensor_reduce(
        s_ss[:, NT : 2 * NT], sq[:].rearrange("p (m t) -> p m t", m=NT),
        axis=AX.X, op=ALU.add)

    G = gmask.shape[-1]
    st_ps = pools["psum_small"].tile([G, 2 * NT], F32, tag="st_ps")
    nc.tensor.matmul(st_ps[:], gmask[:], s_ss[:], start=True, stop=True)
    mu_rs = sb.tile([G, 2 * NT], F32, tag="mu_rs")
    # mu = sum/cnt ; var = sumsq/cnt - mu^2 ; rs = 1/sqrt(var+eps)
    nc.vector.tensor_scalar_mul(mu_rs[:, 0:NT], st_ps[:, 0:NT], 1.0 / cnt)
    var = sb.tile([G, NT], F32, tag="var")
    nc.vector.tensor_scalar_mul(var[:], st_ps[:, NT:], 1.0 / cnt)
    mu2 = sb.tile([G, NT], F32, tag="mu2")
    nc.vector.tensor_tensor(mu2[:], mu_rs[:, 0:NT], mu_rs[:, 0:NT], ALU.mult)
    nc.vector.tensor_tensor(var[:], var[:], mu2[:], ALU.subtract)
    sd = sb.tile([G, NT], F32, tag="sd")
    nc.scalar.activation(sd[:], var[:], AF.Sqrt, bias=epscol[0:G, 0:1])
    nc.vector.reciprocal(mu_rs[:, NT:], sd[:])

    bc_ps = pools["psum_small"].tile([128, 2 * NT], F32, tag="bc_ps")
    nc.tensor.matmul(bc_ps[:], gmaskT[:], mu_rs[:], start=True, stop=True)
    mb = sb.tile([128, 2 * NT], F32, tag="mb")
    nc.scalar.copy(mb[:], bc_ps[:])
    # A = rs*gamma ; Bv = beta - mu*A
    a_b = sb.tile([128, 2 * NT], F32, tag="a_b")
    nc.vector.tensor_tensor(a_b[:, 0:NT], mb[:, NT:], gb[:, 0:NT], ALU.mult)
    tmp = sb.tile([128, NT], F32, tag="abtmp")
    nc.vector.tensor_tensor(tmp[:], mb[:, 0:NT], a_b[:, 0:NT], ALU.mult)
    nc.vector.tensor_tensor(a_b[:, NT:], gb[:, NT:], tmp[:], ALU.subtract)

    for m in range(NT):
        nc.scalar.activation(
            out_writes[m], ps[:, bass.ts(m, T)], AF.Relu,
            scale=a_b[:, m : m + 1], bias=a_b[:, NT + m : NT + m + 1])


def _build_tables(nc, tbl, tin, nlev):
    """Full sparse max-table on tbl [128, 2, nlev*tin, d] (level 0 = data).

    The position axis is packed d-wide over channel tiles; level k entry t
    = max(data[t .. t+2^k-1]); valid width tin - 2^k + 1, tails garbage.
    """
    v = tbl.rearrange("p h (l t) d -> p h l t d", l=nlev)
    for k in range(1, nlev):
        d1, w = 2 ** (k - 1), tin - 2 ** k + 1
        nc.vector.tensor_tensor(
            v[:, :, k, 0:w, :],
            v[:, :, k - 1, 0:w, :],
            v[:, :, k - 1, d1 : d1 + w, :],
            ALU.max)


def _pool_query(tc, pools, tbl, idx, em, ones1b, njj, tin, nlev, out):
    """2 anchors per proposal per half, d-packed gather, max, mask empties.

    tbl [128, 2, nlev*tin, njj]; out [128, 2, njj, N_SEG] (= [128, j, n]).
    """
    nc = tc.nc
    gout = pools["gout"].tile([128, 2, NANCH, N_SEG, njj], F32, tag=f"gout{tin}")
    for h in range(2):
        nc.gpsimd.ap_gather(
            gout[:, h, :, :, :],
            tbl[:, h, :, :],
            idx[:, h * 8 : (h + 1) * 8],
            channels=128, num_elems=nlev * tin, d=njj, num_idxs=NANCH * N_SEG)
    m1 = pools["scratch"].tile([128, 2, njj, N_SEG], F32, tag=f"pm1_{tin}")
    # max over the 2 anchors, transposing (n, jj) -> (jj, n)
    nc.vector.tensor_tensor(
        m1[:],
        gout[:, :, 0, :, :].rearrange("p h n j -> p h j n"),
        gout[:, :, 1, :, :].rearrange("p h n j -> p h j n"),
        ALU.max)
    # zero empty segments: broadcast [1, 2*njj*N] bf16 mask to 128 partitions
    mps = pools["psum_small"].tile([128, 8 * N_SEG], F32, tag="mps")
    nw = 2 * njj * N_SEG
    nc.tensor.matmul(mps[:, 0:nw], ones1b[:], em[:], start=True, stop=True)
    nc.vector.tensor_tensor(
        out[:], m1[:],
        mps[:, 0:nw].rearrange("p (h j n) -> p h j n", h=2, j=njj),
        ALU.mult)


def _cast_hilo(nc, pools, src, name):
    """fp32 SBUF tensor -> (hi, lo) bf16 tensors of the same shape."""
    shp = list(src.shape)
    hi = pools["acts"].tile(shp, BF16, tag=name + "_h")
    lo = pools["acts"].tile(shp, BF16, tag=name + "_l")
    nc.scalar.copy(hi[:], src[:])
    nc.vector.tensor_tensor(lo[:], src[:], hi[:], ALU.subtract)
    return hi, lo


def _build_nc():
    nc = bacc.Bacc("TRN2", target_bir_lowering=False, debug=False,
                   enable_asserts=False, num_devices=B)

    din = {}
    def dram_in(name, shape, dtype=F32):
        din[name] = nc.dram_tensor(name, list(shape), dtype,
                                   kind="ExternalInput").ap()
        return din[name]

    x0 = dram_in("x0", (C, T))
    flf = dram_in("flf", (C, TF))
    conf = dram_in("conf", (CCONF, T))
    for nm, kr, co_ in [("cur", C + 1, C), ("lr", C + 1, 2 * C),
                        ("roi", C + 1, C), ("prop", 4 * C + CCONF + 1, C)]:
        dram_in(f"wth_{nm}", (kr, co_), BF16)
        dram_in(f"wtl_{nm}", (kr, co_), BF16)
    for nm, sh in [("gb_cur", (128, 8)), ("gb_lr_a", (128, 8)),
                   ("gb_lr_b", (128, 8)), ("gb_roi", (128, 8)),
                   ("gb_prop", (128, 8)), ("gmask16", (128, 8)),
                   ("gmask16T", (8, 128)), ("gmask32", (128, 4)),
                   ("gmask32T", (4, 128))]:
        dram_in(nm, sh)
    dram_in("ones1b", (1, 128), BF16)
    dram_in("onezero64", (2, T), BF16)
    dram_in("idx2", (128, 16), I16)
    dram_in("em2", (1, 8 * N_SEG), BF16)
    dram_in("idxf", (128, 16), I16)
    dram_in("emf", (1, 4 * N_SEG), BF16)

    out_d = nc.dram_tensor("out", [C, T], F32, kind="ExternalOutput").ap()
    feat2_d = nc.dram_tensor("feat2", [2 * C, T], F32, kind="ExternalOutput").ap()

    with tile.TileContext(nc) as tc, contextlib.ExitStack() as ctx:
        pools = {
            "consts": ctx.enter_context(tc.tile_pool(name="consts", bufs=1)),
            "wts": ctx.enter_context(tc.tile_pool(name="wts", bufs=1)),
            "acts": ctx.enter_context(tc.tile_pool(name="acts", bufs=1)),
            "sbuf_small": ctx.enter_context(tc.tile_pool(name="sbs", bufs=2)),
            "scratch": ctx.enter_context(tc.tile_pool(name="scr", bufs=2)),
            "gout": ctx.enter_context(tc.tile_pool(name="gout", bufs=1)),
            "psum": ctx.enter_context(
                tc.tile_pool(name="psum", bufs=4, space="PSUM")),
            "psum_small": ctx.enter_context(
                tc.tile_pool(name="psums", bufs=1, space="PSUM")),
        }
        co, wp, ap_ = pools["consts"], pools["wts"], pools["acts"]

        def load(pool, name, shape, dtype=F32, src_ap=None):
            t = pool.tile(list(shape), dtype, tag=name)
            nc.sync.dma_start(t[:], src_ap if src_ap is not None else din[name][:])
            return t

        GB_CUR = load(co, "gb_cur", (128, 8))
        GB_LR_A = load(co, "gb_lr_a", (128, 8))
        GB_LR_B = load(co, "gb_lr_b", (128, 8))
        GB_ROI = load(co, "gb_roi", (128, 8))
        GB_PROP = load(co, "gb_prop", (128, 8))
        GM16 = load(co, "gmask16", (128, 8))
        GM16T = load(co, "gmask16T", (8, 128))
        GM32 = load(co, "gmask32", (128, 4))
        GM32T = load(co, "gmask32T", (4, 128))
        ONES1B = load(co, "ones1b", (1, 128), BF16)
        IDX2 = load(co, "idx2", (128, 16), I16)
        EM2 = load(co, "em2", (1, 8 * N_SEG), BF16)
        IDXF = load(co, "idxf", (128, 16), I16)
        EMF = load(co, "emf", (1, 4 * N_SEG), BF16)

        zcol = co.tile([128, 1], F32, tag="zcol")
        nc.vector.memset(zcol[:], 0.0)
        epscol = co.tile([8, 1], F32, tag="epscol")
        nc.vector.memset(epscol[:], EPS)
        onesb = co.tile([1, T], BF16, tag="onesb")
        nc.vector.memset(onesb[:], 1.0)

        # ---- activations in
        X0 = load(ap_, "x0", (128, 4, T),
                  src_ap=x0.rearrange("(j p) t -> p j t", p=128))
        X0H, X0L = _cast_hilo(nc, pools, X0, "x0")

        FLFRAW = ap_.tile([128, 4, TF], F32, tag="flfraw")
        nc.sync.dma_start(FLFRAW[:], flf.rearrange("(j p) t -> p j t", p=128))
        # frame tables, position-major packed d=2 per half
        TBLFP = ap_.tile([128, 2, NLEVF * TF, 2], F32, tag="tblfp")
        nc.vector.tensor_copy(
            TBLFP[:, :, 0:TF, :],
            FLFRAW[:].rearrange("p (h j) t -> p h t j", h=2))
        _build_tables(nc, TBLFP, TF, NLEVF)
        PR = ap_.tile([128, 2, 2, T], F32, tag="pr")  # [128, (h jj)=j, n]
        _pool_query(tc, pools, TBLFP, IDXF, EMF, ONES1B, 2, TF, NLEVF, PR)
        PRH, PRL = _cast_hilo(nc, pools, PR, "pr")

        CONF = ap_.tile([128, 4, T], F32, tag="conf_t")
        nc.sync.dma_start(CONF[:, 0:3, :],
                          conf[0 : 3 * 128, :].rearrange("(j p) t -> p j t", p=128))
        nc.sync.dma_start(CONF[0:16, 3, :], conf[3 * 128 :, :])
        CONFH, CONFL = _cast_hilo(nc, pools, CONF, "conf")
        # bias row for conv_prop: ones in hi, zeros in lo (partition 16)
        nc.sync.dma_start(CONFH[16:17, 3, :], din["onezero64"][0:1, :])
        nc.sync.dma_start(CONFL[16:17, 3, :], din["onezero64"][1:2, :])

        # ---- weights (bf16 hi/lo, k-major tiles [128, nkt+1, Cout])
        def load_wt(nm, nkt, cout, kr):
            ts_ = []
            for pre in ("wth", "wtl"):
                dram = din[f"{pre}_{nm}"]
                wt_t = wp.tile([128, nkt + 1, cout], BF16, tag=f"{pre}_{nm}")
                full = min(nkt + 1, (kr) // 128)
                nc.sync.dma_start(
                    wt_t[:, 0:full, :],
                    dram[0 : full * 128, :].rearrange("(k p) o -> p k o", p=128))
                rem = kr - full * 128
                if rem:
                    nc.sync.dma_start(wt_t[0:rem, full, :], dram[full * 128 :, :])
                ts_.append(wt_t)
            return ts_

        WTH_CUR, WTL_CUR = load_wt("cur", 4, C, C + 1)
        WTH_LR, WTL_LR = load_wt("lr", 4, 2 * C, C + 1)
        WTH_ROI, WTL_ROI = load_wt("roi", 4, C, C + 1)
        WTH_PROP, WTL_PROP = load_wt("prop", 19, C, 4 * C + CCONF + 1)

        # ---- conv_cur
        FMS = ap_.tile([128, 4, T], F32, tag="fms")
        xh = [X0H[:, k, :] for k in range(4)]
        xl = [X0L[:, k, :] for k in range(4)]
        _conv_gn_relu(tc, pools, WTH_CUR, WTL_CUR, GB_CUR, GM16, GM16T,
                      xh, xl, 4, 16 * T, [FMS[:, m, :] for m in range(4)],
                      onesb, zcol, epscol)
        FMSH, FMSL = _cast_hilo(nc, pools, FMS, "fms")

        # ---- conv_lr -> feat2, two halves
        FEAT2 = ap_.tile([128, 8, T], F32, tag="feat2")
        _conv_gn_relu(tc, pools, WTH_LR, WTL_LR, GB_LR_A, GM32, GM32T,
                      xh, xl, 4, 32 * T, [FEAT2[:, m, :] for m in range(4)],
                      onesb, zcol, epscol)
        _conv_gn_relu(tc, pools, WTH_LR, WTL_LR, GB_LR_B, GM32, GM32T,
                      xh, xl, 4, 32 * T, [FEAT2[:, 4 + m, :] for m in range(4)],
                      onesb, zcol, epscol, m0=4)
        nc.sync.dma_start(feat2_d.rearrange("(j p) t -> p j t", p=128), FEAT2[:])

        # ---- feat2 pooling (packed d=4 per half)
        TBL2P = ap_.tile([128, 2, NLEV2 * T, 4], F32, tag="tbl2p")
        nc.vector.tensor_copy(
            TBL2P[:, :, 0:T, :],
            FEAT2[:].rearrange("p (h j) t -> p h t j", h=2))
        _build_tables(nc, TBL2P, T, NLEV2)
        PF = ap_.tile([128, 2, 4, T], F32, tag="pf")
        _pool_query(tc, pools, TBL2P, IDX2, EM2, ONES1B, 4, T, NLEV2, PF)
        PFH, PFL = _cast_hilo(nc, pools, PF, "pf")

        # ---- conv_roi on pooled frames
        ROIC = ap_.tile([128, 4, T], F32, tag="roic")
        prh = [PRH[:, k // 2, k % 2, :] for k in range(4)]
        prl = [PRL[:, k // 2, k % 2, :] for k in range(4)]
        _conv_gn_relu(tc, pools, WTH_ROI, WTL_ROI, GB_ROI, GM16, GM16T,
                      prh, prl, 4, 16 * T, [ROIC[:, m, :] for m in range(4)],
                      onesb, zcol, epscol)
        ROICH, ROICL = _cast_hilo(nc, pools, ROIC, "roic")

        # ---- conv_prop on the concat
        rhs_h = ([ROICH[:, k, :] for k in range(4)]
                 + [PFH[:, k // 4, k % 4, :] for k in range(8)]
                 + [FMSH[:, k, :] for k in range(4)]
                 + [CONFH[:, k, :] for k in range(4)])
        rhs_l = ([ROICL[:, k, :] for k in range(4)]
                 + [PFL[:, k // 4, k % 4, :] for k in range(8)]
                 + [FMSL[:, k, :] for k in range(4)]
                 + [CONFL[:, k, :] for k in range(4)])
        OUT = ap_.tile([128, 4, T], F32, tag="out_t")
        _conv_gn_relu(tc, pools, WTH_PROP, WTL_PROP, GB_PROP, GM16, GM16T,
                      rhs_h, rhs_l, 20, 16 * T, [OUT[:, m, :] for m in range(4)],
                      onesb, zcol, epscol, last_k=17)
        nc.sync.dma_start(out_d.rearrange("(j p) t -> p j t", p=128), OUT[:])

    nc.compile()
    return nc


# --------------------------------------------------------------------------
# entry point
# --------------------------------------------------------------------------

def kernel(**inputs):
    if "nc" not in _COMPILED:
        _COMPILED["nc"] = _build_nc()
    nc = _COMPILED["nc"]
    in_maps = _host_prep(inputs)
    res = bass_utils.run_bass_kernel_spmd(nc, in_maps, core_ids=list(range(B)))
    outs = res.results
    out = np.stack([outs[b]["out"] for b in range(B)], axis=0)
    feat2 = np.stack([outs[b]["feat2"] for b in range(B)], axis=0)
    return out.astype(np.float32), feat2.astype(np.float32)


# revision 13
# speedup vs baseline: 2.7376x; 1.1962x over previous
"""Trainium2 Bass kernel for nn_CoarsePyramid (nms_detection).

Data-parallel over batch: B=8 -> 8 NeuronCores, one batch element each.

Per-core pipeline (C=512, T=64, TF=256, CCONF=400, GROUPS=32):
  fm_short = CGR(feature, w_cur)            [512, 64]
  feat2    = CGR(feature, w_lr)             [1024, 64]   (also an output)
  prop_feature = boundary_pool(feat2, segments)          [1024, 64]
  prop_roi = CGR(boundary_pool(flf, frame_segments), w_roi)  [512, 64]
  out = CGR(cat(prop_roi, prop_feature, fm_short, conf), w_prop)  [512, 64]

conv1x1: PE matmuls in bf16 hi/lo split (x ~ xh+xl, w ~ wh+wl; psum +=
wh*xh + wh*xl + wl*xh, fp32 accumulate; ~1e-5 rel err) — ~4x faster than
native fp32 matmul on TRN2. Weights pre-transposed on host and shipped as
two bf16 tensors (same total bytes as fp32). Bias rides as an extra
contraction row (K=1 two-pass matmuls against a bf16 ones row; for
conv_prop it sits inside the K=17 conf tail tile).
GroupNorm: per-channel sum (DVE segmented reduce) + sum of squares (ACT
Square + DVE reduce) from PSUM, group-summed/broadcast via tiny fp32 PE
matmuls with 0/1 masks, applied fused with ReLU via ACT(Relu, scale, bias).
Boundary max pooling: full sparse max-table (levels 0..log2(Tin)) built by
DVE shifted-max in a position-major layout packed d-wide over channel
tiles; queries = 2 idempotent anchors per segment, gathered by GPSIMD
ap_gather (one call per half, 128 host-precomputed int16 indices); empty
segments zeroed via a broadcast 0/1 mask.
"""

import contextlib

import numpy as np
import ml_dtypes

import concourse.bass as bass
import concourse.bacc as bacc
import concourse.tile as tile
import concourse.mybir as mybir
from concourse import bass_utils

B, C, T, TF, CCONF = 8, 512, 64, 256, 400
GROUPS, EPS = 32, 1e-5
F32 = mybir.dt.float32
BF16 = mybir.dt.bfloat16
I16 = mybir.dt.int16
AF = mybir.ActivationFunctionType
ALU = mybir.AluOpType
AX = mybir.AxisListType

N_SEG = 64
NLEV2 = 7   # levels 0..6 for Tin=64
NLEVF = 9   # levels 0..8 for Tin=256
NANCH = 2
NT = 4      # output tiles per conv call (Cout=512 per call)

_COMPILED = {}
BF = ml_dtypes.bfloat16


# --------------------------------------------------------------------------
# host-side input prep
# --------------------------------------------------------------------------

def _hi_lo(a):
    hi = a.astype(BF)
    lo = (a - hi.astype(np.float32)).astype(BF)
    return hi, lo


def _wt_pad(w, b):
    """[Cout, Cin] weight + [Cout] bias -> hi/lo bf16 [Cin+1, Cout]."""
    wt = np.concatenate([w.T, b[None, :]], axis=0).astype(np.float32)
    hi, lo = _hi_lo(wt)
    return np.ascontiguousarray(hi), np.ascontiguousarray(lo)


def _gb(g, be):
    """gamma/beta [512] -> [128, 8]: cols [0:4] gamma tiles, [4:8] beta."""
    gt = g.reshape(NT, 128).T
    bt = be.reshape(NT, 128).T
    return np.concatenate([gt, bt], axis=1).astype(np.float32).copy()


def _pool_idx_mask(seg, tin, njj):
    """2-anchor sparse-table gather indices + empty mask.

    Returns idx [128, 16] int16 (two per-half wrapped blocks of
    NANCH*N_SEG indices into that half's [nlev*tin] table) and mask
    [1, 2*njj*N_SEG] bf16 (0 for empty segments), (half, jj)-major.
    """
    s = np.clip(np.floor(seg), 0, tin - 1).astype(np.int64)  # [N, 4]
    idx_h, msk = [], []
    for h in range(2):
        lo, hi = s[:, 2 * h], s[:, 2 * h + 1]
        ln = hi - lo + 1
        ok = ln >= 1
        ln_c = np.maximum(ln, 1)
        k = np.floor(np.log2(ln_c)).astype(np.int64)  # 2^k <= len
        step = 2 ** k
        a0 = k * tin + lo
        a1 = k * tin + np.maximum(hi - step + 1, 0)
        idx = np.stack([a0, a1], axis=0)              # [NANCH, N]
        idx_h.append(np.where(ok[None, :], idx, 0).reshape(-1))
        msk.extend([ok.astype(np.float32)] * njj)
    idx_flat = np.concatenate(idx_h)                  # [2*NANCH*N]
    wrapped = idx_flat.reshape(2, NANCH * N_SEG // 16, 16)
    wrapped = np.transpose(wrapped, (2, 0, 1)).reshape(16, -1)
    idx16 = np.tile(wrapped, (8, 1)).astype(np.int16).copy()  # [128, 16]
    mask = np.concatenate(msk)[None, :].astype(BF).copy()     # [1, 2*njj*N]
    return idx16, mask


def _host_prep(inputs):
    f = {k: np.asarray(v) for k, v in inputs.items()}
    shared = {}
    for nm in ("cur", "lr", "roi", "prop"):
        hi, lo = _wt_pad(f[f"w_{nm}"], f[f"b_{nm}"])
        shared[f"wth_{nm}"], shared[f"wtl_{nm}"] = hi, lo
    shared["gb_cur"] = _gb(f["g_cur"], f["be_cur"])
    shared["gb_lr_a"] = _gb(f["g_lr"][:512], f["be_lr"][:512])
    shared["gb_lr_b"] = _gb(f["g_lr"][512:], f["be_lr"][512:])
    shared["gb_roi"] = _gb(f["g_roi"], f["be_roi"])
    shared["gb_prop"] = _gb(f["g_prop"], f["be_prop"])
    p = np.arange(128)
    gm16 = (p[:, None] // 16 == np.arange(8)[None, :]).astype(np.float32)
    gm32 = (p[:, None] // 32 == np.arange(4)[None, :]).astype(np.float32)
    gmt = np.zeros((8, 256), np.float32)
    gmt[:, 0:128] = gm16.T
    gmt[0:4, 128:256] = gm32.T
    shared["gmt"] = gmt
    cf = np.concatenate(
        [shared.pop("gb_cur"), shared.pop("gb_lr_a"), shared.pop("gb_lr_b"),
         shared.pop("gb_roi"), shared.pop("gb_prop"), gm16, gm32], axis=1)

    in_maps = []
    for b in range(B):
        m = dict(shared)
        x0 = f["feature"][b].reshape(4, 128, T).transpose(1, 0, 2).reshape(128, -1)
        flf = f["frame_level_feature"][b].reshape(4, 128, TF)
        flf = flf.transpose(1, 0, 2).reshape(128, -1)
        confp = np.zeros((512, T), np.float32)
        confp[:CCONF] = f["conf_result_feature"][b]
        confp = confp.reshape(4, 128, T).transpose(1, 0, 2).reshape(128, -1)
        m["constf"] = np.concatenate(
            [cf, x0, flf, confp], axis=1).astype(np.float32).copy()
        idx2, em2 = _pool_idx_mask(f["segments"][b], T, 4)
        idxf, emf = _pool_idx_mask(f["frame_segments"][b], TF, 2)
        m["idxs"] = np.concatenate([idx2, idxf], axis=1).copy()
        cb = np.zeros((1, 1024), np.float32)
        cb[0, 0:128] = 1.0
        cb[0, 128:192] = 1.0   # ones64
        # 192:256 zeros64
        cb[0, 256:768] = em2[0].astype(np.float32)
        cb[0, 768:1024] = emf[0].astype(np.float32)
        m["constb"] = cb.astype(BF).copy()
        in_maps.append(m)
    return in_maps


# --------------------------------------------------------------------------
# device kernel
# --------------------------------------------------------------------------

def _conv_gn_relu(tc, pools, wth, wtl, gb, gmask, gmaskT, rhs_h, rhs_l, nkt,
                  cnt, out_writes, onesb, zcol, epscol, m0=0, last_k=None,
                  kmap=None):
    """bf16 hi/lo conv1x1 (+bias) -> GroupNorm -> ReLU for NT=4 out tiles.

    wth/wtl: SBUF bf16 [128, nkt(+1), Cout_total]; rhs_h/rhs_l: per-k bf16
    [*, T] APs. If last_k is None, bias = K=1 two-pass matmul (wt tile nkt,
    partition 0) against onesb; else the final k-tile has K=last_k rows
    with the bias row included (rhs row last_k-1 is ones in rhs_h and
    zero in rhs_l). cnt: elements per group.
    """
    nc = tc.nc
    sb = pools["sbuf_small"]
    ps = pools["psum"].tile([128, NT * T], F32, tag="conv_ps")
    if kmap is None:
        kmap = list(range(nkt))
    for m in range(NT):
        out_ap = ps[:, bass.ts(m, T)]
        for k in range(nkt):
            kw = kmap[k]
            kk = 128 if (last_k is None or kw < nkt - 1) else last_k
            wh = wth[0:kk, kw, bass.ts(m0 + m, 128)]
            wl = wtl[0:kk, kw, bass.ts(m0 + m, 128)]
            last = last_k is not None and k == nkt - 1
            nc.tensor.matmul(out_ap, wh, rhs_h[k][0:kk, :],
                             start=(k == 0), stop=False)
            nc.tensor.matmul(out_ap, wh, rhs_l[k][0:kk, :],
                             start=False, stop=False)
            nc.tensor.matmul(out_ap, wl, rhs_h[k][0:kk, :],
                             start=False, stop=last)
        if last_k is None:
            nc.tensor.matmul(out_ap, wth[0:1, nkt, bass.ts(m0 + m, 128)],
                             onesb[0:1, :], start=False, stop=False)
            nc.tensor.matmul(out_ap, wtl[0:1, nkt, bass.ts(m0 + m, 128)],
                             onesb[0:1, :], start=False, stop=True)

    # per-channel sum + sum of squares
    s_ss = sb.tile([128, 2 * NT], F32, tag="s_ss")
    nc.vector.tensor_reduce(
        s_ss[:, 0:NT], ps[:].rearrange("p (m t) -> p m t", m=NT),
        axis=AX.X, op=ALU.add)
    sq = pools["scratch"].tile([128, NT * T], F32, tag="sq")
    nc.scalar.activation(sq[:], ps[:], AF.Square, bias=zcol[:, 0:1])
    nc.vector.tensor_reduce(
        s_ss[:, NT : 2 * NT], sq[:].rearrange("p (m t) -> p m t", m=NT),
        axis=AX.X, op=ALU.add)

    G = gmask.shape[-1]
    st_ps = pools["psum_small"].tile([G, 2 * NT], F32, tag="st_ps")
    nc.tensor.matmul(st_ps[:], gmask[:], s_ss[:], start=True, stop=True)
    mu_rs = sb.tile([G, 2 * NT], F32, tag="mu_rs")
    # mu = sum/cnt ; var = sumsq/cnt - mu^2 ; rs = 1/sqrt(var+eps)
    nc.vector.tensor_scalar_mul(mu_rs[:, 0:NT], st_ps[:, 0:NT], 1.0 / cnt)
    var = sb.tile([G, NT], F32, tag="var")
    nc.vector.tensor_scalar_mul(var[:], st_ps[:, NT:], 1.0 / cnt)
    mu2 = sb.tile([G, NT], F32, tag="mu2")
    nc.vector.tensor_tensor(mu2[:], mu_rs[:, 0:NT], mu_rs[:, 0:NT], ALU.mult)
    nc.vector.tensor_tensor(var[:], var[:], mu2[:], ALU.subtract)
    sd = sb.tile([G, NT], F32, tag="sd")
    nc.scalar.activation(sd[:], var[:], AF.Sqrt, bias=epscol[0:G, 0:1])
    nc.vector.reciprocal(mu_rs[:, NT:], sd[:])

    bc_ps = pools["psum_small"].tile([128, 2 * NT], F32, tag="bc_ps")
    nc.tensor.matmul(bc_ps[:], gmaskT[:], mu_rs[:], start=True, stop=True)
    mb = sb.tile([128, 2 * NT], F32, tag="mb")
    nc.scalar.copy(mb[:], bc_ps[:])
    # A = rs*gamma ; Bv = beta - mu*A
    a_b = sb.tile([128, 2 * NT], F32, tag="a_b")
    nc.vector.tensor_tensor(a_b[:, 0:NT], mb[:, NT:], gb[:, 0:NT], ALU.mult)
    tmp = sb.tile([128, NT], F32, tag="abtmp")
    nc.vector.tensor_tensor(tmp[:], mb[:, 0:NT], a_b[:, 0:NT], ALU.mult)
    nc.vector.tensor_tensor(a_b[:, NT:], gb[:, NT:], tmp[:], ALU.subtract)

    for m in range(NT):
        nc.scalar.activation(
            out_writes[m], ps[:, bass.ts(m, T)], AF.Relu,
            scale=a_b[:, m : m + 1], bias=a_b[:, NT + m : NT + m + 1])


def _build_tables(eng, tbl, tin, nlev):
    """Full sparse max-table on tbl [128, 2, nlev*tin, d] (level 0 = data).

    The position axis is packed d-wide over channel tiles; level k entry t
    = max(data[t .. t+2^k-1]); valid width tin - 2^k + 1, tails garbage.
    """
    v = tbl.rearrange("p h (l t) d -> p h l t d", l=nlev)
    for k in range(1, nlev):
        d1, w = 2 ** (k - 1), tin - 2 ** k + 1
        eng.tensor_tensor(
            v[:, :, k, 0:w, :],
            v[:, :, k - 1, 0:w, :],
            v[:, :, k - 1, d1 : d1 + w, :],
            ALU.max)


def _pool_query(tc, pools, tbl, idx, em, ones1b, njj, tin, nlev, out, nh=2):
    """2 anchors per proposal per half, d-packed gather, max, mask empties.

    tbl [128, nh, nlev*tin, njj]; out [128, nh, njj, N_SEG] (= [128, j, n]).
    """
    nc = tc.nc
    gout = pools["gout"].tile([128, nh, NANCH, N_SEG, njj], F32,
                              tag=f"gout{tin}")
    for h in range(nh):
        nc.gpsimd.ap_gather(
            gout[:, h, :, :, :],
            tbl[:, h, :, :],
            idx[:, h * 8 : (h + 1) * 8],
            channels=128, num_elems=nlev * tin, d=njj, num_idxs=NANCH * N_SEG)
    m1 = pools["scratch"].tile([128, nh, njj, N_SEG], F32, tag=f"pm1_{tin}")
    # max over the 2 anchors, transposing (n, jj) -> (jj, n)
    nc.vector.tensor_tensor(
        m1[:],
        gout[:, :, 0, :, :].rearrange("p h n j -> p h j n"),
        gout[:, :, 1, :, :].rearrange("p h n j -> p h j n"),
        ALU.max)
    # zero empty segments: broadcast [1, nh*njj*N] bf16 mask to 128 partitions
    mps = pools["psum_small"].tile([128, 8 * N_SEG], F32, tag="mps")
    nw = nh * njj * N_SEG
    nc.tensor.matmul(mps[:, 0:nw], ones1b[:], em[:], start=True, stop=True)
    nc.vector.tensor_tensor(
        out[:], m1[:],
        mps[:, 0:nw].rearrange("p (h j n) -> p h j n", h=nh, j=njj),
        ALU.mult)


def _cast_hilo(nc, pools, src, name):
    """fp32 SBUF tensor -> (hi, lo) bf16 tensors of the same shape."""
    shp = list(src.shape)
    hi = pools["acts"].tile(shp, BF16, tag=name + "_h")
    lo = pools["acts"].tile(shp, BF16, tag=name + "_l")
    nc.scalar.copy(hi[:], src[:])
    nc.vector.tensor_tensor(lo[:], src[:], hi[:], ALU.subtract)
    return hi, lo


def _build_nc():
    nc = bacc.Bacc("TRN2", target_bir_lowering=False, debug=False,
                   enable_asserts=False, num_devices=B)

    din = {}
    def dram_in(name, shape, dtype=F32):
        din[name] = nc.dram_tensor(name, list(shape), dtype,
                                   kind="ExternalInput").ap()
        return din[name]

    NCF = 52 + 4 * T + 4 * TF + 4 * T
    dram_in("constf", (128, NCF))
    dram_in("constb", (1, 1024), BF16)
    dram_in("idxs", (128, 32), I16)
    dram_in("gmt", (8, 256))
    for nm, kr, co_ in [("cur", C + 1, C), ("lr", C + 1, 2 * C),
                        ("roi", C + 1, C), ("prop", 4 * C + CCONF + 1, C)]:
        dram_in(f"wth_{nm}", (kr, co_), BF16)
        dram_in(f"wtl_{nm}", (kr, co_), BF16)

    out_d = nc.dram_tensor("out", [C, T], F32, kind="ExternalOutput").ap()
    feat2_d = nc.dram_tensor("feat2", [2 * C, T], F32, kind="ExternalOutput").ap()

    with tile.TileContext(nc) as tc, contextlib.ExitStack() as ctx:
        pools = {
            "consts": ctx.enter_context(tc.tile_pool(name="consts", bufs=1)),
            "wts": ctx.enter_context(tc.tile_pool(name="wts", bufs=1)),
            "acts": ctx.enter_context(tc.tile_pool(name="acts", bufs=1)),
            "sbuf_small": ctx.enter_context(tc.tile_pool(name="sbs", bufs=2)),
            "scratch": ctx.enter_context(tc.tile_pool(name="scr", bufs=2)),
            "gout": ctx.enter_context(tc.tile_pool(name="gout", bufs=1)),
            "psum": ctx.enter_context(
                tc.tile_pool(name="psum", bufs=4, space="PSUM")),
            "psum_small": ctx.enter_context(
                tc.tile_pool(name="psums", bufs=1, space="PSUM")),
        }
        co, wp, ap_ = pools["consts"], pools["wts"], pools["acts"]

        def load(pool, name, shape, dtype=F32, src_ap=None):
            t = pool.tile(list(shape), dtype, tag=name)
            nc.sync.dma_start(t[:], src_ap if src_ap is not None else din[name][:])
            return t

        CONSTF = load(co, "constf", (128, 52 + 4 * T + 4 * TF + 4 * T))
        CONSTB = load(co, "constb", (1, 1024), BF16)
        IDXS = load(co, "idxs", (128, 32), I16)
        GMT = load(co, "gmt", (8, 256))

        # ---- weights (bf16 hi/lo, k-major tiles [128, nkt+1, Cout]);
        # queued right behind the const blob so convs can start early
        def load_wt(nm, nkt, cout, kr):
            ts_ = []
            for pre in ("wth", "wtl"):
                dram = din[f"{pre}_{nm}"]
                wt_t = wp.tile([128, nkt + 1, cout], BF16, tag=f"{pre}_{nm}")
                full = min(nkt + 1, (kr) // 128)
                nc.sync.dma_start(
                    wt_t[:, 0:full, :],
                    dram[0 : full * 128, :].rearrange("(k p) o -> p k o", p=128))
                rem = kr - full * 128
                if rem:
                    nc.sync.dma_start(wt_t[0:rem, full, :], dram[full * 128 :, :])
                ts_.append(wt_t)
            return ts_

        WTH_CUR, WTL_CUR = load_wt("cur", 4, C, C + 1)
        WTH_LR, WTL_LR = load_wt("lr", 4, 2 * C, C + 1)
        WTH_ROI, WTL_ROI = load_wt("roi", 4, C, C + 1)
        WTH_PROP, WTL_PROP = load_wt("prop", 19, C, 4 * C + CCONF + 1)

        GB_CUR = CONSTF[:, 0:8]
        GB_LR_A = CONSTF[:, 8:16]
        GB_LR_B = CONSTF[:, 16:24]
        GB_ROI = CONSTF[:, 24:32]
        GB_PROP = CONSTF[:, 32:40]
        GM16 = CONSTF[:, 40:48]
        GM32 = CONSTF[:, 48:52]
        GM16T = GMT[:, 0:128]
        GM32T = GMT[0:4, 128:256]
        X0 = CONSTF[:, 52 : 52 + 4 * T].rearrange("p (j t) -> p j t", j=4)
        FLFRAW = CONSTF[:, 52 + 4 * T : 52 + 4 * T + 4 * TF].rearrange(
            "p (j t) -> p j t", j=4)
        CONF = CONSTF[:, 52 + 4 * T + 4 * TF :].rearrange("p (j t) -> p j t", j=4)
        ONES1B = CONSTB[:, 0:128]
        IDX2 = IDXS[:, 0:16]
        IDXF = IDXS[:, 16:32]
        EM2 = CONSTB[:, 256:768]
        EMF = CONSTB[:, 768:1024]

        zcol = co.tile([128, 1], F32, tag="zcol")
        nc.vector.memset(zcol[:], 0.0)
        epscol = co.tile([8, 1], F32, tag="epscol")
        nc.vector.memset(epscol[:], EPS)
        onesb = co.tile([1, T], BF16, tag="onesb")
        nc.vector.memset(onesb[:], 1.0)

        X0H, X0L = _cast_hilo(nc, pools, X0, "x0")

        # frame tables, position-major packed d=2 per half; built on GPSIMD
        # to keep the DVE free for conv GroupNorm stats
        TBLFP = ap_.tile([128, 2, NLEVF * TF, 2], F32, tag="tblfp")
        nc.vector.tensor_copy(
            TBLFP[:, :, 0:TF, :],
            FLFRAW[:].rearrange("p (h j) t -> p h t j", h=2))
        _build_tables(nc.vector, TBLFP, TF, NLEVF)
        PR = ap_.tile([128, 2, 2, T], F32, tag="pr")  # [128, (h jj)=j, n]
        _pool_query(tc, pools, TBLFP, IDXF, EMF, ONES1B, 2, TF, NLEVF, PR)
        PRH, PRL = _cast_hilo(nc, pools, PR, "pr")

        CONFH, CONFL = _cast_hilo(nc, pools, CONF, "conf")
        # bias row for conv_prop: ones in hi, zeros in lo (partition 16)
        nc.sync.dma_start(CONFH[16:17, 3, :], din["constb"][0:1, 128:192])
        nc.sync.dma_start(CONFL[16:17, 3, :], din["constb"][0:1, 192:256])

        # ---- conv_cur
        FMS = ap_.tile([128, 4, T], F32, tag="fms")
        xh = [X0H[:, k, :] for k in range(4)]
        xl = [X0L[:, k, :] for k in range(4)]
        _conv_gn_relu(tc, pools, WTH_CUR, WTL_CUR, GB_CUR, GM16, GM16T,
                      xh, xl, 4, 16 * T, [FMS[:, m, :] for m in range(4)],
                      onesb, zcol, epscol)
        FMSH, FMSL = _cast_hilo(nc, pools, FMS, "fms")

        # ---- conv_lr -> feat2, two halves
        FEAT2 = ap_.tile([128, 8, T], F32, tag="feat2")
        _conv_gn_relu(tc, pools, WTH_LR, WTL_LR, GB_LR_A, GM32, GM32T,
                      xh, xl, 4, 32 * T, [FEAT2[:, m, :] for m in range(4)],
                      onesb, zcol, epscol)
        _conv_gn_relu(tc, pools, WTH_LR, WTL_LR, GB_LR_B, GM32, GM32T,
                      xh, xl, 4, 32 * T, [FEAT2[:, 4 + m, :] for m in range(4)],
                      onesb, zcol, epscol, m0=4)
        nc.sync.dma_start(feat2_d.rearrange("(j p) t -> p j t", p=128), FEAT2[:])

        # ---- feat2 pooling (packed d=4 per half, per-half pipelines)
        TBL2P = ap_.tile([128, 2, NLEV2 * T, 4], F32, tag="tbl2p")
        PF = ap_.tile([128, 2, 4, T], F32, tag="pf")
        for h in range(2):
            nc.vector.tensor_copy(
                TBL2P[:, h, 0:T, :],
                FEAT2[:, 4 * h : 4 * h + 4, :].rearrange("p j t -> p t j"))
            _build_tables(nc.vector, TBL2P[:, h : h + 1, :, :], T, NLEV2)
            _pool_query(tc, pools, TBL2P[:, h : h + 1, :, :],
                        IDX2[:, 8 * h : 8 * h + 8],
                        EM2[:, 256 * h : 256 * h + 256],
                        ONES1B, 4, T, NLEV2, PF[:, h : h + 1, :, :], nh=1)
        PFH, PFL = _cast_hilo(nc, pools, PF, "pf")

        # ---- conv_roi on pooled frames
        ROIC = ap_.tile([128, 4, T], F32, tag="roic")
        prh = [PRH[:, k // 2, k % 2, :] for k in range(4)]
        prl = [PRL[:, k // 2, k % 2, :] for k in range(4)]
        _conv_gn_relu(tc, pools, WTH_ROI, WTL_ROI, GB_ROI, GM16, GM16T,
                      prh, prl, 4, 16 * T, [ROIC[:, m, :] for m in range(4)],
                      onesb, zcol, epscol)
        ROICH, ROICL = _cast_hilo(nc, pools, ROIC, "roic")

        # ---- conv_prop on the concat
        # k-order: PF-dependent tiles last so conv_prop overlaps the
        # feat2 gathers; weight k-tile indices are permuted to match.
        korder = [0, 1, 2, 3, 12, 13, 14, 15, 16, 17, 18, 19, 4, 5, 6, 7,
                  8, 9, 10, 11]
        rh = ([ROICH[:, k, :] for k in range(4)]
              + [PFH[:, k // 4, k % 4, :] for k in range(8)]
              + [FMSH[:, k, :] for k in range(4)]
              + [CONFH[:, k, :] for k in range(4)])
        rl = ([ROICL[:, k, :] for k in range(4)]
              + [PFL[:, k // 4, k % 4, :] for k in range(8)]
              + [FMSL[:, k, :] for k in range(4)]
              + [CONFL[:, k, :] for k in range(4)])
        rhs_h = [rh[k] for k in korder]
        rhs_l = [rl[k] for k in korder]
        OUT = ap_.tile([128, 4, T], F32, tag="out_t")
        _conv_gn_relu(tc, pools, WTH_PROP, WTL_PROP, GB_PROP, GM16, GM16T,
                      rhs_h, rhs_l, 20, 16 * T, [OUT[:, m, :] for m in range(4)],
                      onesb, zcol, epscol, last_k=17, kmap=korder)
        nc.sync.dma_start(out_d.rearrange("(j p) t -> p j t", p=128), OUT[:])

    nc.compile()
    return nc


# --------------------------------------------------------------------------
# entry point
# --------------------------------------------------------------------------

def kernel(**inputs):
    if "nc" not in _COMPILED:
        _COMPILED["nc"] = _build_nc()
    nc = _COMPILED["nc"]
    in_maps = _host_prep(inputs)
    res = bass_utils.run_bass_kernel_spmd(nc, in_maps, core_ids=list(range(B)))
    outs = res.results
    out = np.stack([outs[b]["out"] for b in range(B)], axis=0)
    feat2 = np.stack([outs[b]["feat2"] for b in range(B)], axis=0)
    return out.astype(np.float32), feat2.astype(np.float32)


# revision 14
# speedup vs baseline: 2.7837x; 1.0168x over previous
"""Trainium2 Bass kernel for nn_CoarsePyramid (nms_detection).

Data-parallel over batch: B=8 -> 8 NeuronCores, one batch element each.

Per-core pipeline (C=512, T=64, TF=256, CCONF=400, GROUPS=32):
  fm_short = CGR(feature, w_cur)            [512, 64]
  feat2    = CGR(feature, w_lr)             [1024, 64]   (also an output)
  prop_feature = boundary_pool(feat2, segments)          [1024, 64]
  prop_roi = CGR(boundary_pool(flf, frame_segments), w_roi)  [512, 64]
  out = CGR(cat(prop_roi, prop_feature, fm_short, conf), w_prop)  [512, 64]

conv1x1: PE matmuls in bf16 hi/lo split (x ~ xh+xl, w ~ wh+wl; psum +=
wh*xh + wh*xl + wl*xh, fp32 accumulate; ~1e-5 rel err) — ~4x faster than
native fp32 matmul on TRN2. Weights pre-transposed on host and shipped as
two bf16 tensors (same total bytes as fp32). Bias rides as an extra
contraction row (K=1 two-pass matmuls against a bf16 ones row; for
conv_prop it sits inside the K=17 conf tail tile).
GroupNorm: per-channel sum (DVE segmented reduce) + sum of squares (ACT
Square + DVE reduce) from PSUM, group-summed/broadcast via tiny fp32 PE
matmuls with 0/1 masks, applied fused with ReLU via ACT(Relu, scale, bias).
Boundary max pooling: full sparse max-table (levels 0..log2(Tin)) built by
DVE shifted-max in a position-major layout packed d-wide over channel
tiles; queries = 2 idempotent anchors per segment, gathered by GPSIMD
ap_gather (one call per half, 128 host-precomputed int16 indices); empty
segments zeroed via a broadcast 0/1 mask.
"""

import contextlib

import numpy as np
import ml_dtypes

import concourse.bass as bass
import concourse.bacc as bacc
import concourse.tile as tile
import concourse.mybir as mybir
from concourse import bass_utils

B, C, T, TF, CCONF = 8, 512, 64, 256, 400
GROUPS, EPS = 32, 1e-5
F32 = mybir.dt.float32
BF16 = mybir.dt.bfloat16
I16 = mybir.dt.int16
AF = mybir.ActivationFunctionType
ALU = mybir.AluOpType
AX = mybir.AxisListType

N_SEG = 64
NLEV2 = 7   # levels 0..6 for Tin=64
NLEVF = 9   # levels 0..8 for Tin=256
NANCH = 2
NT = 4      # output tiles per conv call (Cout=512 per call)

_COMPILED = {}
BF = ml_dtypes.bfloat16


# --------------------------------------------------------------------------
# host-side input prep
# --------------------------------------------------------------------------

def _hi_lo(a):
    hi = a.astype(BF)
    lo = (a - hi.astype(np.float32)).astype(BF)
    return hi, lo


def _wt_pad(w, b):
    """[Cout, Cin] weight + [Cout] bias -> hi/lo bf16 [Cin+1, Cout]."""
    wt = np.concatenate([w.T, b[None, :]], axis=0).astype(np.float32)
    hi, lo = _hi_lo(wt)
    return np.ascontiguousarray(hi), np.ascontiguousarray(lo)


def _gb(g, be):
    """gamma/beta [512] -> [128, 8]: cols [0:4] gamma tiles, [4:8] beta."""
    gt = g.reshape(NT, 128).T
    bt = be.reshape(NT, 128).T
    return np.concatenate([gt, bt], axis=1).astype(np.float32).copy()


def _pool_idx_mask(seg, tin, njj):
    """2-anchor sparse-table gather indices + empty mask.

    Returns idx [128, 16] int16 (two per-half wrapped blocks of
    NANCH*N_SEG indices into that half's [nlev*tin] table) and mask
    [1, 2*njj*N_SEG] bf16 (0 for empty segments), (half, jj)-major.
    """
    s = np.clip(np.floor(seg), 0, tin - 1).astype(np.int64)  # [N, 4]
    idx_h, msk = [], []
    for h in range(2):
        lo, hi = s[:, 2 * h], s[:, 2 * h + 1]
        ln = hi - lo + 1
        ok = ln >= 1
        ln_c = np.maximum(ln, 1)
        k = np.floor(np.log2(ln_c)).astype(np.int64)  # 2^k <= len
        step = 2 ** k
        a0 = k * tin + lo
        a1 = k * tin + np.maximum(hi - step + 1, 0)
        idx = np.stack([a0, a1], axis=0)              # [NANCH, N]
        idx_h.append(np.where(ok[None, :], idx, 0).reshape(-1))
        msk.extend([ok.astype(np.float32)] * njj)
    idx_flat = np.concatenate(idx_h)                  # [2*NANCH*N]
    wrapped = idx_flat.reshape(2, NANCH * N_SEG // 16, 16)
    wrapped = np.transpose(wrapped, (2, 0, 1)).reshape(16, -1)
    idx16 = np.tile(wrapped, (8, 1)).astype(np.int16).copy()  # [128, 16]
    mask = np.concatenate(msk)[None, :].astype(BF).copy()     # [1, 2*njj*N]
    return idx16, mask


def _host_prep(inputs):
    f = {k: np.asarray(v) for k, v in inputs.items()}
    shared = {}
    for nm in ("cur", "lr", "roi", "prop"):
        hi, lo = _wt_pad(f[f"w_{nm}"], f[f"b_{nm}"])
        shared[f"wth_{nm}"], shared[f"wtl_{nm}"] = hi, lo
    shared["gb_cur"] = _gb(f["g_cur"], f["be_cur"])
    shared["gb_lr_a"] = _gb(f["g_lr"][:512], f["be_lr"][:512])
    shared["gb_lr_b"] = _gb(f["g_lr"][512:], f["be_lr"][512:])
    shared["gb_roi"] = _gb(f["g_roi"], f["be_roi"])
    shared["gb_prop"] = _gb(f["g_prop"], f["be_prop"])
    p = np.arange(128)
    gm16 = (p[:, None] // 16 == np.arange(8)[None, :]).astype(np.float32)
    gm32 = (p[:, None] // 32 == np.arange(4)[None, :]).astype(np.float32)
    gmt = np.zeros((8, 256), np.float32)
    gmt[:, 0:128] = gm16.T
    gmt[0:4, 128:256] = gm32.T
    shared["gmt"] = gmt
    cf = np.concatenate(
        [shared.pop("gb_cur"), shared.pop("gb_lr_a"), shared.pop("gb_lr_b"),
         shared.pop("gb_roi"), shared.pop("gb_prop"), gm16, gm32], axis=1)

    in_maps = []
    for b in range(B):
        m = dict(shared)
        x0 = f["feature"][b].reshape(4, 128, T).transpose(1, 0, 2).reshape(128, -1)
        flf = f["frame_level_feature"][b].reshape(4, 128, TF)
        flf = flf.transpose(1, 0, 2).reshape(128, -1)
        confp = np.zeros((512, T), np.float32)
        confp[:CCONF] = f["conf_result_feature"][b]
        confp = confp.reshape(4, 128, T).transpose(1, 0, 2).reshape(128, -1)
        m["constf"] = np.concatenate(
            [cf, x0, flf, confp], axis=1).astype(np.float32).copy()
        idx2, em2 = _pool_idx_mask(f["segments"][b], T, 4)
        idxf, emf = _pool_idx_mask(f["frame_segments"][b], TF, 2)
        m["idxs"] = np.concatenate([idx2, idxf], axis=1).copy()
        cb = np.zeros((1, 1024), np.float32)
        cb[0, 0:128] = 1.0
        cb[0, 128:192] = 1.0   # ones64
        # 192:256 zeros64
        cb[0, 256:768] = em2[0].astype(np.float32)
        cb[0, 768:1024] = emf[0].astype(np.float32)
        m["constb"] = cb.astype(BF).copy()
        in_maps.append(m)
    return in_maps


# --------------------------------------------------------------------------
# device kernel
# --------------------------------------------------------------------------

def _conv_gn_relu(tc, pools, wth, wtl, gb, gmask, gmaskT, rhs_h, rhs_l, nkt,
                  cnt, out_writes, onesb, zcol, epscol, m0=0, last_k=None,
                  kmap=None):
    """bf16 hi/lo conv1x1 (+bias) -> GroupNorm -> ReLU for NT=4 out tiles.

    wth/wtl: SBUF bf16 [128, nkt(+1), Cout_total]; rhs_h/rhs_l: per-k bf16
    [*, T] APs. If last_k is None, bias = K=1 two-pass matmul (wt tile nkt,
    partition 0) against onesb; else the final k-tile has K=last_k rows
    with the bias row included (rhs row last_k-1 is ones in rhs_h and
    zero in rhs_l). cnt: elements per group.
    """
    nc = tc.nc
    sb = pools["sbuf_small"]
    ps = pools["psum"].tile([128, NT * T], F32, tag="conv_ps")
    if kmap is None:
        kmap = list(range(nkt))
    for m in range(NT):
        out_ap = ps[:, bass.ts(m, T)]
        for k in range(nkt):
            kw = kmap[k]
            kk = 128 if (last_k is None or kw < nkt - 1) else last_k
            wh = wth[0:kk, kw, bass.ts(m0 + m, 128)]
            wl = wtl[0:kk, kw, bass.ts(m0 + m, 128)]
            last = last_k is not None and k == nkt - 1
            nc.tensor.matmul(out_ap, wh, rhs_h[k][0:kk, :],
                             start=(k == 0), stop=False)
            nc.tensor.matmul(out_ap, wh, rhs_l[k][0:kk, :],
                             start=False, stop=False)
            nc.tensor.matmul(out_ap, wl, rhs_h[k][0:kk, :],
                             start=False, stop=last)
        if last_k is None:
            nc.tensor.matmul(out_ap, wth[0:1, nkt, bass.ts(m0 + m, 128)],
                             onesb[0:1, :], start=False, stop=False)
            nc.tensor.matmul(out_ap, wtl[0:1, nkt, bass.ts(m0 + m, 128)],
                             onesb[0:1, :], start=False, stop=True)

    # per-channel sum + sum of squares
    s_ss = sb.tile([128, 2 * NT], F32, tag="s_ss")
    nc.vector.tensor_reduce(
        s_ss[:, 0:NT], ps[:].rearrange("p (m t) -> p m t", m=NT),
        axis=AX.X, op=ALU.add)
    sq = pools["scratch"].tile([128, NT * T], F32, tag="sq")
    nc.scalar.activation(sq[:], ps[:], AF.Square, bias=zcol[:, 0:1])
    nc.vector.tensor_reduce(
        s_ss[:, NT : 2 * NT], sq[:].rearrange("p (m t) -> p m t", m=NT),
        axis=AX.X, op=ALU.add)

    G = gmask.shape[-1]
    st_ps = pools["psum_small"].tile([G, 2 * NT], F32, tag="st_ps")
    nc.tensor.matmul(st_ps[:], gmask[:], s_ss[:], start=True, stop=True)
    mu_rs = sb.tile([G, 2 * NT], F32, tag="mu_rs")
    # mu = sum/cnt ; var = sumsq/cnt - mu^2 ; rs = 1/sqrt(var+eps)
    nc.vector.tensor_scalar_mul(mu_rs[:, 0:NT], st_ps[:, 0:NT], 1.0 / cnt)
    var = sb.tile([G, NT], F32, tag="var")
    nc.vector.tensor_scalar_mul(var[:], st_ps[:, NT:], 1.0 / cnt)
    mu2 = sb.tile([G, NT], F32, tag="mu2")
    nc.vector.tensor_tensor(mu2[:], mu_rs[:, 0:NT], mu_rs[:, 0:NT], ALU.mult)
    nc.vector.tensor_tensor(var[:], var[:], mu2[:], ALU.subtract)
    sd = sb.tile([G, NT], F32, tag="sd")
    nc.scalar.activation(sd[:], var[:], AF.Sqrt, bias=epscol[0:G, 0:1])
    nc.vector.reciprocal(mu_rs[:, NT:], sd[:])

    bc_ps = pools["psum_small"].tile([128, 2 * NT], F32, tag="bc_ps")
    nc.tensor.matmul(bc_ps[:], gmaskT[:], mu_rs[:], start=True, stop=True)
    mb = sb.tile([128, 2 * NT], F32, tag="mb")
    nc.scalar.copy(mb[:], bc_ps[:])
    # A = rs*gamma ; Bv = beta - mu*A
    a_b = sb.tile([128, 2 * NT], F32, tag="a_b")
    nc.vector.tensor_tensor(a_b[:, 0:NT], mb[:, NT:], gb[:, 0:NT], ALU.mult)
    tmp = sb.tile([128, NT], F32, tag="abtmp")
    nc.vector.tensor_tensor(tmp[:], mb[:, 0:NT], a_b[:, 0:NT], ALU.mult)
    nc.vector.tensor_tensor(a_b[:, NT:], gb[:, NT:], tmp[:], ALU.subtract)

    for m in range(NT):
        nc.scalar.activation(
            out_writes[m], ps[:, bass.ts(m, T)], AF.Relu,
            scale=a_b[:, m : m + 1], bias=a_b[:, NT + m : NT + m + 1])


def _build_tables(eng, tbl, tin, nlev):
    """Full sparse max-table on tbl [128, 2, nlev*tin, d] (level 0 = data).

    The position axis is packed d-wide over channel tiles; level k entry t
    = max(data[t .. t+2^k-1]); valid width tin - 2^k + 1, tails garbage.
    """
    v = tbl.rearrange("p h (l t) d -> p h l t d", l=nlev)
    for k in range(1, nlev):
        d1, w = 2 ** (k - 1), tin - 2 ** k + 1
        eng.tensor_tensor(
            v[:, :, k, 0:w, :],
            v[:, :, k - 1, 0:w, :],
            v[:, :, k - 1, d1 : d1 + w, :],
            ALU.max)


def _pool_query(tc, pools, tbl, idx, em, ones1b, njj, tin, nlev, out, nh=2):
    """2 anchors per proposal per half, d-packed gather, max, mask empties.

    tbl [128, nh, nlev*tin, njj]; out [128, nh, njj, N_SEG] (= [128, j, n]).
    """
    nc = tc.nc
    gout = pools["gout"].tile([128, nh, NANCH, N_SEG, njj], F32,
                              tag=f"gout{tin}")
    for h in range(nh):
        nc.gpsimd.ap_gather(
            gout[:, h, :, :, :],
            tbl[:, h, :, :],
            idx[:, h * 8 : (h + 1) * 8],
            channels=128, num_elems=nlev * tin, d=njj, num_idxs=NANCH * N_SEG)
    m1 = pools["scratch"].tile([128, nh, njj, N_SEG], F32, tag=f"pm1_{tin}")
    # max over the 2 anchors, transposing (n, jj) -> (jj, n)
    nc.vector.tensor_tensor(
        m1[:],
        gout[:, :, 0, :, :].rearrange("p h n j -> p h j n"),
        gout[:, :, 1, :, :].rearrange("p h n j -> p h j n"),
        ALU.max)
    # zero empty segments with the pre-broadcast 0/1 mask (em: [128, ...])
    nw = nh * njj * N_SEG
    nc.vector.tensor_tensor(
        out[:], m1[:],
        em[:, 0:nw].rearrange("p (h j n) -> p h j n", h=nh, j=njj),
        ALU.mult)


def _cast_hilo(nc, pools, src, name):
    """fp32 SBUF tensor -> (hi, lo) bf16 tensors of the same shape."""
    shp = list(src.shape)
    hi = pools["acts"].tile(shp, BF16, tag=name + "_h")
    lo = pools["acts"].tile(shp, BF16, tag=name + "_l")
    nc.scalar.copy(hi[:], src[:])
    nc.vector.tensor_tensor(lo[:], src[:], hi[:], ALU.subtract)
    return hi, lo


def _build_nc():
    nc = bacc.Bacc("TRN2", target_bir_lowering=False, debug=False,
                   enable_asserts=False, num_devices=B)

    din = {}
    def dram_in(name, shape, dtype=F32):
        din[name] = nc.dram_tensor(name, list(shape), dtype,
                                   kind="ExternalInput").ap()
        return din[name]

    NCF = 52 + 4 * T + 4 * TF + 4 * T
    dram_in("constf", (128, NCF))
    dram_in("constb", (1, 1024), BF16)
    dram_in("idxs", (128, 32), I16)
    dram_in("gmt", (8, 256))
    for nm, kr, co_ in [("cur", C + 1, C), ("lr", C + 1, 2 * C),
                        ("roi", C + 1, C), ("prop", 4 * C + CCONF + 1, C)]:
        dram_in(f"wth_{nm}", (kr, co_), BF16)
        dram_in(f"wtl_{nm}", (kr, co_), BF16)

    out_d = nc.dram_tensor("out", [C, T], F32, kind="ExternalOutput").ap()
    feat2_d = nc.dram_tensor("feat2", [2 * C, T], F32, kind="ExternalOutput").ap()

    with tile.TileContext(nc) as tc, contextlib.ExitStack() as ctx:
        pools = {
            "consts": ctx.enter_context(tc.tile_pool(name="consts", bufs=1)),
            "wts": ctx.enter_context(tc.tile_pool(name="wts", bufs=1)),
            "acts": ctx.enter_context(tc.tile_pool(name="acts", bufs=1)),
            "sbuf_small": ctx.enter_context(tc.tile_pool(name="sbs", bufs=2)),
            "scratch": ctx.enter_context(tc.tile_pool(name="scr", bufs=2)),
            "gout": ctx.enter_context(tc.tile_pool(name="gout", bufs=1)),
            "psum": ctx.enter_context(
                tc.tile_pool(name="psum", bufs=4, space="PSUM")),
            "psum_small": ctx.enter_context(
                tc.tile_pool(name="psums", bufs=1, space="PSUM")),
        }
        co, wp, ap_ = pools["consts"], pools["wts"], pools["acts"]

        def load(pool, name, shape, dtype=F32, src_ap=None):
            t = pool.tile(list(shape), dtype, tag=name)
            nc.sync.dma_start(t[:], src_ap if src_ap is not None else din[name][:])
            return t

        CONSTF = load(co, "constf", (128, 52 + 4 * T + 4 * TF + 4 * T))
        CONSTB = load(co, "constb", (1, 1024), BF16)
        IDXS = load(co, "idxs", (128, 32), I16)
        GMT = load(co, "gmt", (8, 256))

        # ---- weights (bf16 hi/lo, k-major tiles [128, nkt+1, Cout]);
        # queued right behind the const blob so convs can start early
        def load_wt(nm, nkt, cout, kr):
            ts_ = []
            for pre in ("wth", "wtl"):
                dram = din[f"{pre}_{nm}"]
                wt_t = wp.tile([128, nkt + 1, cout], BF16, tag=f"{pre}_{nm}")
                full = min(nkt + 1, (kr) // 128)
                nc.sync.dma_start(
                    wt_t[:, 0:full, :],
                    dram[0 : full * 128, :].rearrange("(k p) o -> p k o", p=128))
                rem = kr - full * 128
                if rem:
                    nc.sync.dma_start(wt_t[0:rem, full, :], dram[full * 128 :, :])
                ts_.append(wt_t)
            return ts_

        WTH_CUR, WTL_CUR = load_wt("cur", 4, C, C + 1)
        WTH_LR, WTL_LR = load_wt("lr", 4, 2 * C, C + 1)
        WTH_ROI, WTL_ROI = load_wt("roi", 4, C, C + 1)
        WTH_PROP, WTL_PROP = load_wt("prop", 19, C, 4 * C + CCONF + 1)

        GB_CUR = CONSTF[:, 0:8]
        GB_LR_A = CONSTF[:, 8:16]
        GB_LR_B = CONSTF[:, 16:24]
        GB_ROI = CONSTF[:, 24:32]
        GB_PROP = CONSTF[:, 32:40]
        GM16 = CONSTF[:, 40:48]
        GM32 = CONSTF[:, 48:52]
        GM16T = GMT[:, 0:128]
        GM32T = GMT[0:4, 128:256]
        X0 = CONSTF[:, 52 : 52 + 4 * T].rearrange("p (j t) -> p j t", j=4)
        FLFRAW = CONSTF[:, 52 + 4 * T : 52 + 4 * T + 4 * TF].rearrange(
            "p (j t) -> p j t", j=4)
        CONF = CONSTF[:, 52 + 4 * T + 4 * TF :].rearrange("p (j t) -> p j t", j=4)
        ONES1B = CONSTB[:, 0:128]
        IDX2 = IDXS[:, 0:16]
        IDXF = IDXS[:, 16:32]
        EM2 = CONSTB[:, 256:768]
        EMF = CONSTB[:, 768:1024]

        zcol = co.tile([128, 1], F32, tag="zcol")
        nc.vector.memset(zcol[:], 0.0)
        epscol = co.tile([8, 1], F32, tag="epscol")
        nc.vector.memset(epscol[:], EPS)
        onesb = co.tile([1, T], BF16, tag="onesb")
        nc.vector.memset(onesb[:], 1.0)

        # broadcast the empty-segment masks to all partitions once, early
        # (depends only on inputs; keeps the PE stream unblocked later)
        mps = pools["psum_small"].tile([128, 8 * N_SEG], F32, tag="mps")
        nc.tensor.matmul(mps[:, 0:512], ONES1B[:], EM2[:], start=True, stop=True)
        MASK2 = co.tile([128, 512], F32, tag="mask2")
        nc.scalar.copy(MASK2[:], mps[:, 0:512])
        mpsf = pools["psum_small"].tile([128, 8 * N_SEG], F32, tag="mps")
        nc.tensor.matmul(mpsf[:, 0:256], ONES1B[:], EMF[:], start=True, stop=True)
        MASKF = co.tile([128, 256], F32, tag="maskf")
        nc.scalar.copy(MASKF[:], mpsf[:, 0:256])

        X0H, X0L = _cast_hilo(nc, pools, X0, "x0")

        # frame tables, position-major packed d=2 per half; built on GPSIMD
        # to keep the DVE free for conv GroupNorm stats
        TBLFP = ap_.tile([128, 2, NLEVF * TF, 2], F32, tag="tblfp")
        nc.vector.tensor_copy(
            TBLFP[:, :, 0:TF, :],
            FLFRAW[:].rearrange("p (h j) t -> p h t j", h=2))
        _build_tables(nc.vector, TBLFP, TF, NLEVF)
        PR = ap_.tile([128, 2, 2, T], F32, tag="pr")  # [128, (h jj)=j, n]
        _pool_query(tc, pools, TBLFP, IDXF, MASKF, ONES1B, 2, TF, NLEVF, PR)
        PRH, PRL = _cast_hilo(nc, pools, PR, "pr")

        CONFH, CONFL = _cast_hilo(nc, pools, CONF, "conf")
        # bias row for conv_prop: ones in hi, zeros in lo (partition 16)
        nc.sync.dma_start(CONFH[16:17, 3, :], din["constb"][0:1, 128:192])
        nc.sync.dma_start(CONFL[16:17, 3, :], din["constb"][0:1, 192:256])

        # ---- conv_cur
        FMS = ap_.tile([128, 4, T], F32, tag="fms")
        xh = [X0H[:, k, :] for k in range(4)]
        xl = [X0L[:, k, :] for k in range(4)]
        _conv_gn_relu(tc, pools, WTH_CUR, WTL_CUR, GB_CUR, GM16, GM16T,
                      xh, xl, 4, 16 * T, [FMS[:, m, :] for m in range(4)],
                      onesb, zcol, epscol)
        FMSH, FMSL = _cast_hilo(nc, pools, FMS, "fms")

        # ---- conv_lr -> feat2, two halves
        FEAT2 = ap_.tile([128, 8, T], F32, tag="feat2")
        _conv_gn_relu(tc, pools, WTH_LR, WTL_LR, GB_LR_A, GM32, GM32T,
                      xh, xl, 4, 32 * T, [FEAT2[:, m, :] for m in range(4)],
                      onesb, zcol, epscol)
        _conv_gn_relu(tc, pools, WTH_LR, WTL_LR, GB_LR_B, GM32, GM32T,
                      xh, xl, 4, 32 * T, [FEAT2[:, 4 + m, :] for m in range(4)],
                      onesb, zcol, epscol, m0=4)
        nc.sync.dma_start(feat2_d.rearrange("(j p) t -> p j t", p=128), FEAT2[:])

        # ---- conv_roi on pooled frames
        ROIC = ap_.tile([128, 4, T], F32, tag="roic")
        prh = [PRH[:, k // 2, k % 2, :] for k in range(4)]
        prl = [PRL[:, k // 2, k % 2, :] for k in range(4)]
        _conv_gn_relu(tc, pools, WTH_ROI, WTL_ROI, GB_ROI, GM16, GM16T,
                      prh, prl, 4, 16 * T, [ROIC[:, m, :] for m in range(4)],
                      onesb, zcol, epscol)
        ROICH, ROICL = _cast_hilo(nc, pools, ROIC, "roic")

        # ---- conv_prop on the concat
        # ---- feat2 pooling (packed d=4 per half, per-half pipelines)
        TBL2P = ap_.tile([128, 2, NLEV2 * T, 4], F32, tag="tbl2p")
        PF = ap_.tile([128, 2, 4, T], F32, tag="pf")
        for h in range(2):
            nc.vector.tensor_copy(
                TBL2P[:, h, 0:T, :],
                FEAT2[:, 4 * h : 4 * h + 4, :].rearrange("p j t -> p t j"))
            _build_tables(nc.vector, TBL2P[:, h : h + 1, :, :], T, NLEV2)
            _pool_query(tc, pools, TBL2P[:, h : h + 1, :, :],
                        IDX2[:, 8 * h : 8 * h + 8],
                        MASK2[:, 256 * h : 256 * h + 256],
                        ONES1B, 4, T, NLEV2, PF[:, h : h + 1, :, :], nh=1)
        PFH, PFL = _cast_hilo(nc, pools, PF, "pf")

        # k-order: PF-dependent tiles last so conv_prop overlaps the
        # feat2 gathers; weight k-tile indices are permuted to match.
        korder = [0, 1, 2, 3, 12, 13, 14, 15, 16, 17, 18, 19, 4, 5, 6, 7,
                  8, 9, 10, 11]
        rh = ([ROICH[:, k, :] for k in range(4)]
              + [PFH[:, k // 4, k % 4, :] for k in range(8)]
              + [FMSH[:, k, :] for k in range(4)]
              + [CONFH[:, k, :] for k in range(4)])
        rl = ([ROICL[:, k, :] for k in range(4)]
              + [PFL[:, k // 4, k % 4, :] for k in range(8)]
              + [FMSL[:, k, :] for k in range(4)]
              + [CONFL[:, k, :] for k in range(4)])
        rhs_h = [rh[k] for k in korder]
        rhs_l = [rl[k] for k in korder]
        OUT = ap_.tile([128, 4, T], F32, tag="out_t")
        _conv_gn_relu(tc, pools, WTH_PROP, WTL_PROP, GB_PROP, GM16, GM16T,
                      rhs_h, rhs_l, 20, 16 * T, [OUT[:, m, :] for m in range(4)],
                      onesb, zcol, epscol, last_k=17, kmap=korder)
        nc.sync.dma_start(out_d.rearrange("(j p) t -> p j t", p=128), OUT[:])

    nc.compile()
    return nc


# --------------------------------------------------------------------------
# entry point
# --------------------------------------------------------------------------

def kernel(**inputs):
    if "nc" not in _COMPILED:
        _COMPILED["nc"] = _build_nc()
    nc = _COMPILED["nc"]
    in_maps = _host_prep(inputs)
    res = bass_utils.run_bass_kernel_spmd(nc, in_maps, core_ids=list(range(B)))
    outs = res.results
    out = np.stack([outs[b]["out"] for b in range(B)], axis=0)
    feat2 = np.stack([outs[b]["feat2"] for b in range(B)], axis=0)
    return out.astype(np.float32), feat2.astype(np.float32)
